# revision 11
# baseline (speedup 1.0000x reference)
"""Trainium2 Bass kernel for nn_CausalWanSelfAttention (sparse_attention).

Strategy: tensor-parallel over heads across 8 NeuronCores. Each core owns
2 of the 16 heads and processes all 1760 tokens:
  - fused QKV projection (bf16 matmuls, PSUM-accumulated over 16 k-tiles)
  - RMS-norm: local sum-of-squares, ONE tiny AllReduce for the full-2048
    channel statistic, ln/exp-based rsqrt on ACT
  - rope applied BEFORE normalization (they commute): k transposes happen
    pre-AllReduce; the k-norm scale is folded into the exp's per-partition
    scale; the q-norm scale is a per-partition tensor_scalar multiply; the
    per-channel gains are folded into the host-built rope tables
  - attention in transposed layout: scores^T = kw^T(T) @ rq^T, exp on ACT,
    PV accumulation on PE; softmax denominator via DVE accumulation + PE
    ones-matmul partition reduce + DVE approx reciprocal + PE rank-1
    broadcast matmul; normalize with a DVE multiply
  - output projection, bf16 AllToAll of o^T so each core emits 220 rows;
    the final y-wave is split into even/odd-head halves so the even half
    overlaps the last AllToAll
Host side (free): input slicing/transposition/bf16 casts, rope freq
tables (per-partition contiguous), final concat + output bias.
"""
import sys

for _p in ("/opt/trn_rl_repo", "/root/.axon_site/_ro/trn_rl_repo"):
    if _p not in sys.path:
        sys.path.append(_p)

import numpy as np
import ml_dtypes

import concourse.bass as bass
import concourse.bacc as bacc
import concourse.mybir as mybir
from concourse import bass_isa
from concourse.tile import TileContext
from concourse.bass_utils import run_bass_kernel_spmd
from concourse.masks import make_identity

BF16 = ml_dtypes.bfloat16
S, DIM, NH, D = 1760, 2048, 16, 128
TW = 3520          # attention window length
WIN0 = 2640        # cache rows [2640:4400] form the first half of the window
NCORES, HPC = 8, 2
CH = HPC * D       # 256 channels per core
EPS = 1e-6
SCALE = 1.0 / float(np.sqrt(D))
S_OUT = S // NCORES  # 220 rows of output per core

S_TILES = [(i * 128, min(128, S - i * 128)) for i in range((S + 127) // 128)]
NT = len(S_TILES)  # 14
# window t-tiles: cache part [0,1760) then new part [1760,3520)
T_TILES = ([("c", j, off, sz) for j, (off, sz) in enumerate(S_TILES)]
           + [("n", j, off + S, sz) for j, (off, sz) in enumerate(S_TILES)])
SJ = [(0, 880), (880, 880)]  # attention s-chunks

_CACHE = {}


def _emit(nc):
    dt = mybir.dt
    BF, F32 = dt.bfloat16, dt.float32
    A = mybir.ActivationFunctionType
    Op = mybir.AluOpType
    core_ids = list(range(NCORES))

    xT = nc.declare_dram_parameter("xT", [NT, 128, DIM], BF, isOutput=False)
    wTt = nc.declare_dram_parameter("wTt", [128, 16 * 768], BF, isOutput=False)
    woTt = nc.declare_dram_parameter("woTt", [128, 16 * 2048], BF,
                                     isOutput=False)
    ckT = nc.declare_dram_parameter("ckT", [HPC, D, S], BF, isOutput=False)
    cv = nc.declare_dram_parameter("cv", [HPC, 128, NT * D], BF, isOutput=False)
    # rope tables with gains folded in, tile-major per-partition contiguous
    ftab = [nc.declare_dram_parameter(f"ftab{i}", [128, NT * 256], BF,
                                      isOutput=False) for i in range(4)]
    gbd = nc.declare_dram_parameter("gb", [1, 3 * CH], F32, isOutput=False)
    y_out = nc.declare_dram_parameter("y", [S_OUT, DIM], F32, isOutput=True)

    ss_in = nc.dram_tensor("ss_in", [128, 28], F32)
    ss_out = nc.dram_tensor("ss_out", [128, 28], F32, addr_space="Shared")
    # o-matrix all-to-all: two waves (s 0:880 and 880:1760) x two heads;
    # each core ends up with o^T columns for its own 110-row slice
    a2a_in = [[nc.dram_tensor(f"a2a_in{w}_{h}", [NCORES, D, 110], BF)
               for h in range(2)] for w in range(2)]
    a2a_out = [[nc.dram_tensor(f"a2a_out{w}_{h}", [NCORES, D, 110], BF)
                for h in range(2)] for w in range(2)]

    from contextlib import ExitStack
    with TileContext(nc) as tc, ExitStack() as stack:
        cpool = stack.enter_context(tc.tile_pool(name="const", bufs=1))
        wpool = stack.enter_context(tc.tile_pool(name="work", bufs=3))
        rqpool = tc.alloc_tile_pool(name="rqp", bufs=1)
        ppool = tc.alloc_tile_pool(name="projp", bufs=1)
        wtpool = tc.alloc_tile_pool(name="wtp", bufs=1)

        # ---- startup DMAs: xt0 + wT chunks first, spread across queues ----
        xt0 = wpool.tile([128, DIM], BF, tag="xt0", bufs=1, name="xt0pre")
        nc.sync.dma_start(out=xt0[:], in_=xT[0])
        gb_row = cpool.tile([1, 3 * CH], F32, tag="gb_row")
        nc.scalar.dma_start(out=gb_row[:], in_=gbd[:])
        wTbig = wtpool.tile([128, 16 * 768], BF, tag="wTbig", name="wTbig")
        wq_trig = [nc.scalar, nc.gpsimd, nc.sync, nc.scalar]
        for c in range(4):
            wq_trig[c].dma_start(out=wTbig[:, 3072 * c:3072 * (c + 1)],
                                 in_=wTt[:, 3072 * c:3072 * (c + 1)])
        wT_sb = [wTbig[:, 768 * kk:768 * (kk + 1)] for kk in range(16)]
        gb_full = cpool.tile([128, 3 * CH], F32, tag="gb_full")
        nc.gpsimd.partition_broadcast(gb_full[:, 0:2 * CH], gb_row[:, 0:2 * CH])
        nc.gpsimd.partition_broadcast(gb_full[:, 2 * CH:3 * CH],
                                      gb_row[:, 2 * CH:3 * CH])
        bqkB = gb_full[:, 0:2 * CH]
        bvB = gb_full[:, 2 * CH:3 * CH]

        ident = cpool.tile([128, 128], BF, tag="ident")
        make_identity(nc, ident[:])
        ones_col = cpool.tile([128, 1], BF, tag="ones_col")
        nc.gpsimd.memset(ones_col[:], 1.0)
        ones_row = cpool.tile([1, 128], BF, tag="ones_row")
        nc.gpsimd.memset(ones_row[:], 1.0)

        # rope tables (gains folded in): frA, fiA, fiB, frB — resident
        ftab_sb = []
        ft_trig = [nc.gpsimd, nc.scalar, nc.gpsimd, nc.scalar]
        for i in range(4):
            t = cpool.tile([128, NT * 256], BF, tag=f"ftab{i}", name=f"ftab{i}")
            ft_trig[i].dma_start(out=t[:], in_=ftab[i][:])
            ftab_sb.append(t)

        # attention constants (cache halves of the window) — loads overlap
        # the projection phase
        kwT_sb = []
        cv_sb = [[], []]
        for hh in range(HPC):
            t = cpool.tile([128, TW], BF, tag=f"kwT{hh}", name=f"kwT{hh}")
            kwT_sb.append(t)
            nc.scalar.dma_start(out=t[:, 0:S], in_=ckT[hh])
            big = cpool.tile([128, NT * D], BF, tag=f"cva{hh}", name=f"cva{hh}")
            nc.gpsimd.dma_start(out=big[:], in_=cv[hh])
            cv_sb[hh] = [big[:, j * D:(j + 1) * D] for j in range(NT)]

        qk_sb, v_sb, rq_store = [], [], {}
        for j in range(NT):
            qk_sb.append(ppool.tile([128, 2 * CH], F32, tag=f"qk{j}",
                                    name=f"qk{j}"))
            v_sb.append(cpool.tile([128, CH], BF, tag=f"v{j}", name=f"v{j}"))

        rqT_sb = [cpool.tile([128, S], BF, tag=f"rqT{hh}", name=f"rqT{hh}")
                  for hh in range(HPC)]
        oT_sb = [cpool.tile([128, S], BF, tag=f"oT{hh}", name=f"oT{hh}")
                 for hh in range(HPC)]

        ss = cpool.tile([128, 28], F32, tag="ss")
        nc.gpsimd.memset(ss[:], 0.0)
        eps_ap = cpool.tile([128, 1], F32, tag="eps_ap")
        nc.gpsimd.memset(eps_ap[:], EPS)

        rs_holder = {}

        def issue_ar():
            nc.scalar.dma_start(out=ss_in[:], in_=ss[:])
            nc.gpsimd.collective_compute(
                "AllReduce", Op.add, replica_groups=[core_ids],
                ins=[ss_in[:]], outs=[ss_out[:]])
            ssg = cpool.tile([128, 28], F32, tag="ssg")
            nc.scalar.dma_start(out=ssg[:], in_=ss_out[:])
            rs_holder["ssg"] = ssg

        def finish_ar():
            ssg = rs_holder["ssg"]
            tmp = wpool.tile([128, 28], F32, tag="rstmp", name="rstmp")
            nc.scalar.activation(tmp[:], ssg[:], A.Ln, scale=1.0 / DIM,
                                 bias=eps_ap[:])
            rqk = cpool.tile([128, 28], F32, tag="rqk", name="rqk")
            nc.scalar.activation(rqk[:], tmp[:], A.Exp, scale=-0.5)
            rs_holder["rs_q"] = rqk[:, 0:14]
            rsk = cpool.tile([128, 14], F32, tag="rsk", name="rsk")
            nc.vector.tensor_scalar_mul(rsk[:], rqk[:, 14:28], SCALE)
            rs_holder["rs_k_s"] = rsk

        # rope on un-normalized q and k (rope commutes with the per-row rms
        # scale); per-channel gains are folded into the ftab tables
        def rope_gain(j, ktr_pool):
            off, sz = S_TILES[j]
            fA = ftab_sb[0][:sz, 256 * j:256 * (j + 1)].rearrange(
                "p (b c) -> p b c", b=4)
            fiA = ftab_sb[1][:sz, 256 * j:256 * (j + 1)].rearrange(
                "p (b c) -> p b c", b=4)
            fiB = ftab_sb[2][:sz, 256 * j:256 * (j + 1)].rearrange(
                "p (b c) -> p b c", b=4)
            frB = ftab_sb[3][:sz, 256 * j:256 * (j + 1)].rearrange(
                "p (b c) -> p b c", b=4)
            q3 = qk_sb[j][:sz, :].rearrange("p (b c) -> p b c", b=4)
            qe, qo = q3[:, :, 0:64], q3[:, :, 64:128]
            rq4 = rqpool.tile([128, 2 * CH], BF, tag=f"rq{j}", name=f"rq{j}")
            r3 = rq4[:sz, :].rearrange("p (b c) -> p b c", b=4)
            t1 = wpool.tile([128, 256], BF, tag="ropet1")
            t2 = wpool.tile([128, 256], BF, tag="ropet2")
            t13 = t1[:sz, :].rearrange("p (b c) -> p b c", b=4)
            t23 = t2[:sz, :].rearrange("p (b c) -> p b c", b=4)
            nc.vector.tensor_mul(t13, qe, fA)
            nc.vector.tensor_mul(t23, qo, fiA)
            nc.vector.tensor_sub(r3[:, :, 0:64], t13, t23)
            t3 = wpool.tile([128, 256], BF, tag="ropet1")
            t4 = wpool.tile([128, 256], BF, tag="ropet2")
            t33 = t3[:sz, :].rearrange("p (b c) -> p b c", b=4)
            t43 = t4[:sz, :].rearrange("p (b c) -> p b c", b=4)
            nc.vector.tensor_mul(t33, qe, fiB)
            nc.vector.tensor_mul(t43, qo, frB)
            nc.vector.tensor_add(r3[:, :, 64:128], t33, t43)
            rq_store[j] = rq4
            # k transposes now (pre-AllReduce): k-norm folds into exp scale
            for hh in range(HPC):
                tp = ktr_pool.tile([128, 128], BF, tag="ktr")
                nc.tensor.transpose(tp[:, :sz],
                                    rq4[:sz, CH + D * hh:CH + D * (hh + 1)],
                                    ident[:sz, :sz])
                nc.scalar.copy(kwT_sb[hh][:, S + off:S + off + sz], tp[:, :sz])

        xt_trig = [nc.sync, nc.gpsimd]

        # ---- phase 1: q/k projection + sum-of-squares; then one AllReduce --
        with tc.tile_pool(name="pj", bufs=2, space="PSUM") as pj:
            for j, (off, sz) in enumerate(S_TILES):
                if j == 0:
                    xt = xt0
                else:
                    xt = wpool.tile([128, DIM], BF, tag=f"xt{j % 2}", bufs=1,
                                    name=f"xt{j}")
                    xt_trig[j % 2].dma_start(out=xt[:], in_=xT[j])
                ps = pj.tile([128, 512], F32, tag="qk")
                for kk in range(16):
                    nc.tensor.matmul(ps[:sz, 0:512], xt[:, 128 * kk:128 * kk + sz],
                                     wT_sb[kk][:, 0:512],
                                     start=(kk == 0), stop=(kk == 15))
                nc.vector.tensor_add(qk_sb[j][:sz, :], ps[:sz, :], bqkB[:sz, :])
                sq = wpool.tile([128, CH], F32, tag="sqscratch", bufs=2)
                nc.scalar.activation(sq[:sz, :], qk_sb[j][:sz, 0:CH], A.Square,
                                     accum_out=ss[:sz, j:j + 1])
                sq2 = wpool.tile([128, CH], F32, tag="sqscratch", bufs=2)
                nc.scalar.activation(sq2[:sz, :], qk_sb[j][:sz, CH:2 * CH],
                                     A.Square, accum_out=ss[:sz, 14 + j:15 + j])
            issue_ar()
            # ---- phase 2: v projection + rope + k transposes (AR in flight) --
            for j, (off, sz) in enumerate(S_TILES):
                xt = wpool.tile([128, DIM], BF, tag=f"xt{j % 2}", bufs=1,
                                name=f"xtv{j}")
                xt_trig[j % 2].dma_start(out=xt[:], in_=xT[j])
                ps = pj.tile([128, CH], F32, tag="v")
                for kk in range(16):
                    nc.tensor.matmul(ps[:sz, :], xt[:, 128 * kk:128 * kk + sz],
                                     wT_sb[kk][:, 512:768],
                                     start=(kk == 0), stop=(kk == 15))
                nc.vector.tensor_add(v_sb[j][:sz, :], ps[:sz, :], bvB[:sz, :])
                rope_gain(j, pj)

        wtpool.release()
        ppool.release()

        # ---- phase 3: q norm-scale + transposes, then attention ----
        with tc.tile_pool(name="pat", bufs=2, space="PSUM") as pat:
            finish_ar()
            rs_q = rs_holder["rs_q"]
            rs_k_s = rs_holder["rs_k_s"]
            for j, (off, sz) in enumerate(S_TILES):
                rqs = wpool.tile([128, CH], BF, tag="rqs", bufs=2,
                                 name=f"rqs{j}")
                nc.vector.tensor_scalar_mul(rqs[:sz, :],
                                            rq_store[j][:sz, 0:CH],
                                            rs_q[:sz, j:j + 1])
                for hh in range(HPC):
                    tp = pat.tile([128, 128], BF, tag="tr")
                    nc.tensor.transpose(tp[:, :sz],
                                        rqs[:sz, D * hh:D * (hh + 1)],
                                        ident[:sz, :sz])
                    nc.vector.tensor_copy(rqT_sb[hh][:, off:off + sz],
                                          tp[:, :sz])
            rqpool.release()
            # output-projection weights: big contiguous chunks on sync+gpsimd
            # (NOT the ACT queue — its triggers would stall attention exps)
            tpool = tc.alloc_tile_pool(name="tailp", bufs=1)
            woTbig = tpool.tile([128, 16 * 2048], BF, tag="woTbig",
                                name="woTbig")
            wo_trig = [nc.sync, nc.gpsimd, nc.sync, nc.gpsimd]
            for c in range(4):
                wo_trig[c].dma_start(out=woTbig[:, 8192 * c:8192 * (c + 1)],
                                     in_=woTt[:, 8192 * c:8192 * (c + 1)])
            woT_sb = [woTbig[:, 2048 * kk:2048 * (kk + 1)] for kk in range(16)]

            att = {}

            def attn_state(hh, jc):
                st = att.get((hh, jc))
                if st is None:
                    o_ps = pat.tile([128, 880], F32, tag="o", bufs=1,
                                    name=f"o{hh}_{jc}")
                    den = wpool.tile([128, 880], BF, tag="den", bufs=2,
                                     name=f"den{hh}_{jc}")
                    pts = {}
                    st = att[(hh, jc)] = (o_ps, den, pts)
                return st

            def attn_sc(hh, jc, tlist):
                jof, jsz = SJ[jc]
                o_ps, den, pts = attn_state(hh, jc)
                for ti in tlist:
                    part, j2, toff, tsz = T_TILES[ti]
                    sc = pat.tile([128, 880], F32, tag="sc")
                    nc.tensor.matmul(
                        sc[:tsz, 0:512], kwT_sb[hh][:, toff:toff + tsz],
                        rqT_sb[hh][:, jof:jof + 512], start=True, stop=True)
                    nc.tensor.matmul(
                        sc[:tsz, 512:880], kwT_sb[hh][:, toff:toff + tsz],
                        rqT_sb[hh][:, jof + 512:jof + 880],
                        start=True, stop=True)
                    pT = wpool.tile([128, 880], BF, tag="pT", bufs=4)
                    esc = (SCALE if part == "c"
                           else rs_k_s[:tsz, j2:j2 + 1])
                    nc.scalar.activation(pT[:tsz, :], sc[:tsz, :], A.Exp,
                                         scale=esc)
                    if ti == 0:
                        nc.vector.tensor_copy(den[:, :], pT[:, :])
                    else:
                        nc.vector.tensor_add(den[:tsz, :], den[:tsz, :],
                                             pT[:tsz, :])
                    pts[ti] = pT

            def attn_pv(hh, jc, tlist):
                o_ps, den, pts = attn_state(hh, jc)
                for ti in tlist:
                    part, j2, toff, tsz = T_TILES[ti]
                    pT = pts.pop(ti)
                    vt = (cv_sb[hh][j2][:tsz, :] if part == "c"
                          else v_sb[j2][:tsz, D * hh:D * (hh + 1)])
                    last = ti == len(T_TILES) - 1
                    nc.tensor.matmul(o_ps[:, 0:512], vt, pT[:tsz, 0:512],
                                     start=(ti == 0), stop=last)
                    nc.tensor.matmul(o_ps[:, 512:880], vt, pT[:tsz, 512:880],
                                     start=(ti == 0), stop=last)

            def attn_finish(hh, jc, mid=None):
                jof, jsz = SJ[jc]
                o_ps, den, pts = att[(hh, jc)]
                # softmax denominator: PE partition-reduce, DVE approx
                # reciprocal, PE rank-1 broadcast, DVE normalize
                redps = pat.tile([128, 880], F32, tag="sc", name=f"red{hh}_{jc}")
                nc.tensor.matmul(redps[0:1, 0:512], ones_col[:, :],
                                 den[:, 0:512], start=True, stop=True)
                nc.tensor.matmul(redps[0:1, 512:880], ones_col[:, :],
                                 den[:, 512:880], start=True, stop=True)
                o_raw = wpool.tile([128, 880], BF, tag="oraw", bufs=2,
                                   name=f"oraw{hh}_{jc}")
                nc.vector.tensor_copy(o_raw[:, :jsz], o_ps[:, :jsz])
                drf = wpool.tile([1, 880], F32, tag="dln", bufs=1,
                                 name=f"dln{hh}_{jc}")
                nc.vector.reciprocal_approx_fast(drf[0:1, :], redps[0:1, :])
                denr = wpool.tile([1, 880], BF, tag="denr", bufs=2,
                                  name=f"denr{hh}_{jc}")
                nc.scalar.copy(denr[0:1, :], drf[0:1, :])
                if mid is not None:
                    mid()
                denb = pat.tile([128, 880], F32, tag="sc", name=f"denb{hh}_{jc}")
                nc.tensor.matmul(denb[:, 0:512], ones_row[:, :],
                                 denr[0:1, 0:512], start=True, stop=True)
                nc.tensor.matmul(denb[:, 512:880], ones_row[:, :],
                                 denr[0:1, 512:880], start=True, stop=True)
                nc.vector.tensor_mul(
                    oT_sb[hh][:, jof:jof + jsz], o_raw[:, :jsz], denb[:, :jsz])

            def emit_a2a(w, hh):
                nc.sync.dma_start(
                    out=a2a_in[w][hh][:].rearrange("d p s -> p d s"),
                    in_=oT_sb[hh][:, 880 * w:880 * (w + 1)]
                        .rearrange("p (d s) -> p d s", s=110))
                nc.gpsimd.collective_compute(
                    "AllToAll", mybir.AluOpType.bypass,
                    replica_groups=[core_ids],
                    ins=[a2a_in[w][hh][:]], outs=[a2a_out[w][hh][:]])

            otr_sb = {}

            def load_otr(w, hh):
                t = tpool.tile([128, 8 * 110], BF, tag=f"otr{w}_{hh}",
                               name=f"otr{w}_{hh}")
                nc.sync.dma_start(
                    out=t[:].rearrange("p (d s) -> p d s", s=110),
                    in_=a2a_out[w][hh][:].rearrange("d p s -> p d s"))
                otr_sb[(w, hh)] = t

            yp_store = {}

            def wave_y_mm(w, nlist, hhs, tags):
                for n in nlist:
                    yp = yp_store.get((w, n))
                    if yp is None:
                        yp = pat.tile([128, 512], F32, tag=tags[n],
                                      name=f"yp{w}_{n}")
                        yp_store[(w, n)] = yp
                    for kk in range(16):
                        src_c, hh = kk // 2, kk % 2
                        if hh not in hhs:
                            continue
                        nc.tensor.matmul(
                            yp[:110, :],
                            otr_sb[(w, hh)][:, 110 * src_c:110 * (src_c + 1)],
                            woT_sb[kk][:, 512 * n:512 * (n + 1)],
                            start=(kk == 0), stop=(kk == 15))

            def wave_y_chunk_out(w, n, yf):
                yp = yp_store[(w, n)]
                nc.scalar.copy(yf[:110, 512 * n:512 * (n + 1)], yp[:110, :])
                nc.sync.dma_start(
                    out=y_out[110 * w:110 * (w + 1), 512 * n:512 * (n + 1)],
                    in_=yf[:110, 512 * n:512 * (n + 1)])

            def wave_y_full(w):
                yf = wpool.tile([128, DIM], F32, tag="yf", bufs=1, name=f"yf{w}")
                for n in range(4):
                    wave_y_mm(w, [n], (0, 1), ["tr"] * 4)
                    wave_y_chunk_out(w, n, yf)

            def wave_y_out(w):
                yf = wpool.tile([128, DIM], F32, tag="yf", bufs=1, name=f"yf{w}")
                for n in range(4):
                    wave_y_chunk_out(w, n, yf)

            TRS = ["tr", "tr", "sc", "sc"]

            # chunk order: (0,0) (1,0) (0,1) (1,1); a2a emitted per
            # (wave, head) as soon as that head's wave chunk finishes
            attn_sc(0, 0, [0, 1])
            attn_pv(0, 0, [0, 1])
            for ti in range(2, 28):
                attn_sc(0, 0, [ti])
                attn_pv(0, 0, [ti])
            attn_sc(1, 0, [0, 1])
            attn_finish(0, 0)
            emit_a2a(0, 0)
            attn_pv(1, 0, [0, 1])
            for ti in range(2, 28):
                attn_sc(1, 0, [ti])
                attn_pv(1, 0, [ti])
            attn_sc(0, 1, [0, 1])
            attn_finish(1, 0)
            emit_a2a(0, 1)
            load_otr(0, 0)
            attn_pv(0, 1, [0, 1])
            for ti in range(2, 28):
                attn_sc(0, 1, [ti])
                attn_pv(0, 1, [ti])
            attn_sc(1, 1, [0, 1])
            attn_finish(0, 1)
            emit_a2a(1, 0)
            load_otr(0, 1)
            attn_pv(1, 1, [0, 1])
            for ti in range(2, 14):
                attn_sc(1, 1, [ti])
                attn_pv(1, 1, [ti])
            wave_y_full(0)
            load_otr(1, 0)
            for ti in range(14, 28):
                attn_sc(1, 1, [ti])
                attn_pv(1, 1, [ti])
            # last chunk: even-head y matmuls overlap the final AllToAll
            attn_finish(1, 1, mid=lambda: wave_y_mm(1, [0], (0,), TRS))
            emit_a2a(1, 1)
            wave_y_mm(1, [1, 2, 3], (0,), TRS)
            load_otr(1, 1)
            wave_y_mm(1, [0, 1, 2, 3], (1,), TRS)
            wave_y_out(1)
        tpool.release()


def _build():
    if "nc" not in _CACHE:
        nc = bacc.Bacc("TRN2", target_bir_lowering=False, debug=False,
                       num_devices=NCORES)
        _emit(nc)
        nc.compile()
        _CACHE["nc"] = nc
    return _CACHE["nc"]


def _make_fcomb(freqs):
    F, H, W = 2, 20, 44
    fr = np.asarray(freqs, np.float32)  # [1024, 64, 2]
    fpart = np.broadcast_to(fr[5:7, None, None, 0:22], (F, H, W, 22, 2))
    hpart = np.broadcast_to(fr[None, 0:H, None, 22:43], (F, H, W, 21, 2))
    wpart = np.broadcast_to(fr[None, None, 0:W, 43:64], (F, H, W, 21, 2))
    return np.concatenate([fpart, hpart, wpart], axis=3).reshape(S, 64, 2)


def _tile_major(a):
    """[S, C] -> [128, NT*C] tile-major (per-partition contiguous)."""
    C = a.shape[1]
    ap = np.zeros((NT * 128, C), np.float32)
    ap[:S] = a
    return np.ascontiguousarray(
        ap.reshape(NT, 128, C).transpose(1, 0, 2).reshape(128, NT * C))


def kernel(x, wq, bq, wk, bk, wv, bv, wo, bo, gq, gk, freqs, cache_k, cache_v):
    x = np.asarray(x, np.float32)
    wq, wk, wv, wo = (np.asarray(a, np.float32) for a in (wq, wk, wv, wo))
    bq, bk, bv, bo = (np.asarray(a, np.float32) for a in (bq, bk, bv, bo))
    gq, gk = np.asarray(gq, np.float32), np.asarray(gk, np.float32)
    cache_k = np.asarray(cache_k, np.float32)
    cache_v = np.asarray(cache_v, np.float32)

    fcomb = _make_fcomb(freqs)  # [S, 64, 2]
    fr_t, fi_t = fcomb[..., 0], fcomb[..., 1]  # [S, 64]
    # pre-tiled x^T: xT[j, p, kk*128+c] = x[128j+c, 128kk+p]
    xp = np.zeros((NT * 128, DIM), np.float32)
    xp[:S] = x[0]
    xT = np.ascontiguousarray(
        xp.reshape(NT, 128, 16, 128).transpose(0, 3, 2, 1).reshape(NT, 128, DIM)
    ).astype(BF16)

    # de-interleave rope channel pairs within each head: [2c] then [2c+1]
    # (applied consistently to wq/wk rows, their biases/gains, and the
    # transposed k-cache, so attention dot products are unchanged)
    perm = np.concatenate([np.arange(0, D, 2), np.arange(1, D, 2)])
    qk_perm = np.concatenate([h * D + perm for h in range(NH)])
    wqp, wkp = wq[qk_perm], wk[qk_perm]
    bqp, bkp = bq[qk_perm], bk[qk_perm]
    gqp, gkp = gq[qk_perm], gk[qk_perm]
    ck_perm = cache_k[0, WIN0:WIN0 + S][:, :, perm]  # [S, NH, D] channel-permuted

    woT_full = np.ascontiguousarray(wo.T).astype(np.float32)  # [DIM, DIM]
    in_maps = []
    for c in range(NCORES):
        hs = slice(CH * c, CH * (c + 1))
        h0 = HPC * c
        wTc = np.concatenate([wqp[hs].T, wkp[hs].T, wv[hs].T], axis=1)
        # tile-major: wTt[p, kk*768+cc] = wTc[128kk+p, cc]
        wTt = np.ascontiguousarray(
            wTc.reshape(16, 128, 768).transpose(1, 0, 2).reshape(128, 16 * 768)
        ).astype(BF16)
        woTt = np.ascontiguousarray(
            woT_full.reshape(16, 128, 2048).transpose(1, 0, 2)
            .reshape(128, 16 * 2048)).astype(BF16)
        ckTc = np.ascontiguousarray(
            ck_perm[:, h0:h0 + HPC, :].transpose(1, 2, 0)
        ).astype(BF16)  # [HPC, D, S]
        # pre-tiled cache-v: cvc[hh, p, j*128+d] = cv_window[128j+p, h, d]
        cw = np.zeros((NT * 128, HPC, D), np.float32)
        cw[:S] = cache_v[0, WIN0:WIN0 + S, h0:h0 + HPC, :]
        cvc = np.ascontiguousarray(
            cw.reshape(NT, 128, HPC, D).transpose(2, 1, 0, 3).reshape(HPC, 128, NT * D)
        ).astype(BF16)
        # rope tables with per-block gains folded in; blocks are
        # (q-h0, q-h1, k-h0, k-h1), each [64 even | 64 odd] channels
        ge = [gqp[hs][0:64], gqp[hs][128:192], gkp[hs][0:64], gkp[hs][128:192]]
        go = [gqp[hs][64:128], gqp[hs][192:256],
              gkp[hs][64:128], gkp[hs][192:256]]
        tabs = []
        for src, gl in ((fr_t, ge), (fi_t, go), (fi_t, ge), (fr_t, go)):
            tab = np.concatenate([src * gl[b][None, :] for b in range(4)],
                                 axis=1)  # [S, 256]
            tabs.append(_tile_major(tab).astype(BF16))
        gb = np.concatenate([bqp[hs], bkp[hs], bv[hs]])
        in_maps.append({
            "xT": xT, "wTt": wTt, "woTt": woTt,
            "ckT": ckTc, "cv": cvc,
            "ftab0": tabs[0], "ftab1": tabs[1], "ftab2": tabs[2],
            "ftab3": tabs[3],
            "gb": np.ascontiguousarray(gb)[None, :].astype(np.float32),
        })

    nc = _build()
    res = run_bass_kernel_spmd(nc, in_maps, list(range(NCORES)))
    _CACHE["last_result"] = res
    # all-to-all layout: core c returns rows [110c:110c+110] and
    # [880+110c:880+110c+110]
    y = np.empty((S, DIM), np.float32)
    for c in range(NCORES):
        yc = res.results[c]["y"]
        y[110 * c:110 * (c + 1)] = yc[:110]
        y[880 + 110 * c:880 + 110 * (c + 1)] = yc[110:]
    return (y + bo[None, :]).reshape(1, S, DIM).astype(np.float32)


# revision 12
# speedup vs baseline: 1.0873x; 1.0873x over previous
"""Trainium2 Bass kernel for nn_CausalWanSelfAttention (sparse_attention).

Strategy: tensor-parallel over heads across 8 NeuronCores. Each core owns
2 of the 16 heads and processes all 1760 tokens:
  - fused QKV projection (bf16 matmuls, PSUM-accumulated over 16 k-tiles)
  - RMS-norm: local sum-of-squares, ONE tiny AllReduce for the full-2048
    channel statistic, ln/exp-based rsqrt on ACT
  - rope applied BEFORE normalization (they commute): k transposes happen
    pre-AllReduce; the k-norm scale is folded into the exp's per-partition
    scale; the q-norm scale is a per-partition tensor_scalar multiply; the
    per-channel gains are folded into the host-built rope tables
  - attention in transposed layout: scores^T = kw^T(T) @ rq^T, exp on ACT,
    PV accumulation on PE; softmax denominator via DVE accumulation + PE
    ones-matmul partition reduce + DVE approx reciprocal + PE rank-1
    broadcast matmul; normalize with a DVE multiply
  - output projection, bf16 AllToAll of o^T so each core emits 220 rows;
    the final y-wave is split into even/odd-head halves so the even half
    overlaps the last AllToAll
Host side (free): input slicing/transposition/bf16 casts, rope freq
tables (per-partition contiguous), final concat + output bias.
"""
import sys

for _p in ("/opt/trn_rl_repo", "/root/.axon_site/_ro/trn_rl_repo"):
    if _p not in sys.path:
        sys.path.append(_p)

import numpy as np
import ml_dtypes

import concourse.bass as bass
import concourse.bacc as bacc
import concourse.mybir as mybir
from concourse import bass_isa
from concourse.tile import TileContext
from concourse.bass_utils import run_bass_kernel_spmd
from concourse.masks import make_identity

BF16 = ml_dtypes.bfloat16
S, DIM, NH, D = 1760, 2048, 16, 128
TW = 3520          # attention window length
WIN0 = 2640        # cache rows [2640:4400] form the first half of the window
NCORES, HPC = 8, 2
CH = HPC * D       # 256 channels per core
EPS = 1e-6
SCALE = 1.0 / float(np.sqrt(D))
S_OUT = S // NCORES  # 220 rows of output per core

S_TILES = [(i * 128, min(128, S - i * 128)) for i in range((S + 127) // 128)]
NT = len(S_TILES)  # 14
# window t-tiles: cache part [0,1760) then new part [1760,3520)
T_TILES = ([("c", j, off, sz) for j, (off, sz) in enumerate(S_TILES)]
           + [("n", j, off + S, sz) for j, (off, sz) in enumerate(S_TILES)])
SJ = [(0, 880), (880, 880)]  # attention s-chunks

_CACHE = {}


def _emit(nc):
    dt = mybir.dt
    BF, F32 = dt.bfloat16, dt.float32
    A = mybir.ActivationFunctionType
    Op = mybir.AluOpType
    core_ids = list(range(NCORES))

    xT = nc.declare_dram_parameter("xT", [NT, 128, DIM], BF, isOutput=False)
    wTt = nc.declare_dram_parameter("wTt", [128, 16 * 768], BF, isOutput=False)
    woTt = nc.declare_dram_parameter("woTt", [128, 16 * 2048], BF,
                                     isOutput=False)
    ckT = nc.declare_dram_parameter("ckT", [HPC, D, S], BF, isOutput=False)
    cv = nc.declare_dram_parameter("cv", [HPC, 128, NT * D], BF, isOutput=False)
    # rope tables with gains folded in, tile-major per-partition contiguous
    ftab = [nc.declare_dram_parameter(f"ftab{i}", [128, NT * 256], BF,
                                      isOutput=False) for i in range(4)]
    gbd = nc.declare_dram_parameter("gb", [1, 3 * CH], F32, isOutput=False)
    y_out = nc.declare_dram_parameter("y", [S_OUT, DIM], F32, isOutput=True)

    ss_in = nc.dram_tensor("ss_in", [128, 28], F32)
    ss_out = nc.dram_tensor("ss_out", [128, 28], F32, addr_space="Shared")
    # o-matrix all-to-all: two waves (s 0:880 and 880:1760) x two heads;
    # each core ends up with o^T columns for its own 110-row slice
    a2a_in = [[nc.dram_tensor(f"a2a_in{w}_{h}", [NCORES, D, 110], BF)
               for h in range(2)] for w in range(2)]
    a2a_out = [[nc.dram_tensor(f"a2a_out{w}_{h}", [NCORES, D, 110], BF)
                for h in range(2)] for w in range(2)]

    from contextlib import ExitStack
    with TileContext(nc) as tc, ExitStack() as stack:
        cpool = stack.enter_context(tc.tile_pool(name="const", bufs=1))
        wpool = stack.enter_context(tc.tile_pool(name="work", bufs=3))
        rqpool = tc.alloc_tile_pool(name="rqp", bufs=1)
        ppool = tc.alloc_tile_pool(name="projp", bufs=1)
        wtpool = tc.alloc_tile_pool(name="wtp", bufs=1)

        # ---- startup DMAs: xt0 + wT chunks first, spread across queues ----
        xt0 = wpool.tile([128, DIM], BF, tag="xt0", bufs=1, name="xt0pre")
        nc.sync.dma_start(out=xt0[:], in_=xT[0])
        gb_row = cpool.tile([1, 3 * CH], F32, tag="gb_row")
        nc.scalar.dma_start(out=gb_row[:], in_=gbd[:])
        wTbig = wtpool.tile([128, 16 * 768], BF, tag="wTbig", name="wTbig")
        wq_trig = [nc.scalar, nc.gpsimd, nc.sync, nc.scalar]
        for c in range(4):
            wq_trig[c].dma_start(out=wTbig[:, 3072 * c:3072 * (c + 1)],
                                 in_=wTt[:, 3072 * c:3072 * (c + 1)])
        wT_sb = [wTbig[:, 768 * kk:768 * (kk + 1)] for kk in range(16)]
        gb_full = cpool.tile([128, 3 * CH], F32, tag="gb_full")
        nc.gpsimd.partition_broadcast(gb_full[:, 0:2 * CH], gb_row[:, 0:2 * CH])
        nc.gpsimd.partition_broadcast(gb_full[:, 2 * CH:3 * CH],
                                      gb_row[:, 2 * CH:3 * CH])
        bqkB = gb_full[:, 0:2 * CH]
        bvB = gb_full[:, 2 * CH:3 * CH]

        ident = cpool.tile([128, 128], BF, tag="ident")
        make_identity(nc, ident[:])
        ones_col = cpool.tile([128, 1], BF, tag="ones_col")
        nc.gpsimd.memset(ones_col[:], 1.0)
        ones_row = cpool.tile([1, 128], BF, tag="ones_row")
        nc.gpsimd.memset(ones_row[:], 1.0)

        # rope tables (gains folded in): frA, fiA, fiB, frB — resident
        ftab_sb = []
        ft_trig = [nc.scalar, nc.scalar, nc.scalar, nc.scalar]
        for i in range(4):
            t = cpool.tile([128, NT * 256], BF, tag=f"ftab{i}", name=f"ftab{i}")
            ft_trig[i].dma_start(out=t[:], in_=ftab[i][:])
            ftab_sb.append(t)

        # attention constants (cache halves of the window) — loads overlap
        # the projection phase
        kwT_sb = []
        cv_sb = [[], []]
        for hh in range(HPC):
            t = cpool.tile([128, TW], BF, tag=f"kwT{hh}", name=f"kwT{hh}")
            kwT_sb.append(t)
            nc.scalar.dma_start(out=t[:, 0:S], in_=ckT[hh])
            big = cpool.tile([128, NT * D], BF, tag=f"cva{hh}", name=f"cva{hh}")
            nc.scalar.dma_start(out=big[:], in_=cv[hh])
            cv_sb[hh] = [big[:, j * D:(j + 1) * D] for j in range(NT)]

        qk_sb, v_sb, rq_store = [], [], {}
        for j in range(NT):
            qk_sb.append(ppool.tile([128, 2 * CH], F32, tag=f"qk{j}",
                                    name=f"qk{j}"))
            v_sb.append(cpool.tile([128, CH], BF, tag=f"v{j}", name=f"v{j}"))

        rqT_sb = [cpool.tile([128, S], BF, tag=f"rqT{hh}", name=f"rqT{hh}")
                  for hh in range(HPC)]
        oT_sb = [cpool.tile([128, S], BF, tag=f"oT{hh}", name=f"oT{hh}")
                 for hh in range(HPC)]

        ss = cpool.tile([128, 28], F32, tag="ss")
        nc.gpsimd.memset(ss[:], 0.0)
        eps_ap = cpool.tile([128, 1], F32, tag="eps_ap")
        nc.gpsimd.memset(eps_ap[:], EPS)

        rs_holder = {}

        def issue_ar():
            nc.scalar.dma_start(out=ss_in[:], in_=ss[:])
            nc.gpsimd.collective_compute(
                "AllReduce", Op.add, replica_groups=[core_ids],
                ins=[ss_in[:]], outs=[ss_out[:]])

        def finish_ar():
            ssg = cpool.tile([128, 28], F32, tag="ssg")
            nc.sync.dma_start(out=ssg[:], in_=ss_out[:])
            tmp = wpool.tile([128, 28], F32, tag="rstmp", name="rstmp")
            nc.scalar.activation(tmp[:], ssg[:], A.Ln, scale=1.0 / DIM,
                                 bias=eps_ap[:])
            rqk = cpool.tile([128, 28], F32, tag="rqk", name="rqk")
            nc.scalar.activation(rqk[:], tmp[:], A.Exp, scale=-0.5)
            rs_holder["rs_q"] = rqk[:, 0:14]
            rsk = cpool.tile([128, 14], F32, tag="rsk", name="rsk")
            nc.vector.tensor_scalar_mul(rsk[:], rqk[:, 14:28], SCALE)
            rs_holder["rs_k_s"] = rsk

        # rope on un-normalized q and k (rope commutes with the per-row rms
        # scale); per-channel gains are folded into the ftab tables
        def rope_gain(j, ktr_pool):
            off, sz = S_TILES[j]
            fA = ftab_sb[0][:sz, 256 * j:256 * (j + 1)].rearrange(
                "p (b c) -> p b c", b=4)
            fiA = ftab_sb[1][:sz, 256 * j:256 * (j + 1)].rearrange(
                "p (b c) -> p b c", b=4)
            fiB = ftab_sb[2][:sz, 256 * j:256 * (j + 1)].rearrange(
                "p (b c) -> p b c", b=4)
            frB = ftab_sb[3][:sz, 256 * j:256 * (j + 1)].rearrange(
                "p (b c) -> p b c", b=4)
            q3 = qk_sb[j][:sz, :].rearrange("p (b c) -> p b c", b=4)
            qe, qo = q3[:, :, 0:64], q3[:, :, 64:128]
            rq4 = rqpool.tile([128, 2 * CH], BF, tag=f"rq{j}", name=f"rq{j}")
            r3 = rq4[:sz, :].rearrange("p (b c) -> p b c", b=4)
            t1 = wpool.tile([128, 256], BF, tag="ropet1")
            t2 = wpool.tile([128, 256], BF, tag="ropet2")
            t13 = t1[:sz, :].rearrange("p (b c) -> p b c", b=4)
            t23 = t2[:sz, :].rearrange("p (b c) -> p b c", b=4)
            nc.vector.tensor_mul(t13, qe, fA)
            nc.vector.tensor_mul(t23, qo, fiA)
            nc.vector.tensor_sub(r3[:, :, 0:64], t13, t23)
            t3 = wpool.tile([128, 256], BF, tag="ropet1")
            t4 = wpool.tile([128, 256], BF, tag="ropet2")
            t33 = t3[:sz, :].rearrange("p (b c) -> p b c", b=4)
            t43 = t4[:sz, :].rearrange("p (b c) -> p b c", b=4)
            nc.vector.tensor_mul(t33, qe, fiB)
            nc.vector.tensor_mul(t43, qo, frB)
            nc.vector.tensor_add(r3[:, :, 64:128], t33, t43)
            rq_store[j] = rq4
            # k transposes now (pre-AllReduce): k-norm folds into exp scale
            for hh in range(HPC):
                tp = ktr_pool.tile([128, 128], BF, tag="ktr")
                nc.tensor.transpose(tp[:, :sz],
                                    rq4[:sz, CH + D * hh:CH + D * (hh + 1)],
                                    ident[:sz, :sz])
                nc.scalar.copy(kwT_sb[hh][:, S + off:S + off + sz], tp[:, :sz])

        xt_trig = [nc.sync, nc.gpsimd]

        # ---- phase 1: q/k projection + sum-of-squares; then one AllReduce --
        with tc.tile_pool(name="pj", bufs=2, space="PSUM") as pj:
            for j, (off, sz) in enumerate(S_TILES):
                if j == 0:
                    xt = xt0
                else:
                    xt = wpool.tile([128, DIM], BF, tag=f"xt{j % 2}", bufs=1,
                                    name=f"xt{j}")
                    xt_trig[j % 2].dma_start(out=xt[:], in_=xT[j])
                ps = pj.tile([128, 512], F32, tag="qk")
                for kk in range(16):
                    nc.tensor.matmul(ps[:sz, 0:512], xt[:, 128 * kk:128 * kk + sz],
                                     wT_sb[kk][:, 0:512],
                                     start=(kk == 0), stop=(kk == 15))
                nc.vector.tensor_add(qk_sb[j][:sz, :], ps[:sz, :], bqkB[:sz, :])
                sq = wpool.tile([128, CH], F32, tag="sqscratch", bufs=2)
                nc.scalar.activation(sq[:sz, :], qk_sb[j][:sz, 0:CH], A.Square,
                                     accum_out=ss[:sz, j:j + 1])
                sq2 = wpool.tile([128, CH], F32, tag="sqscratch", bufs=2)
                nc.scalar.activation(sq2[:sz, :], qk_sb[j][:sz, CH:2 * CH],
                                     A.Square, accum_out=ss[:sz, 14 + j:15 + j])
            issue_ar()
            # ---- phase 2: v projection + rope + k transposes (AR in flight) --
            for j, (off, sz) in enumerate(S_TILES):
                xt = wpool.tile([128, DIM], BF, tag=f"xt{j % 2}", bufs=1,
                                name=f"xtv{j}")
                xt_trig[j % 2].dma_start(out=xt[:], in_=xT[j])
                ps = pj.tile([128, CH], F32, tag="v")
                for kk in range(16):
                    nc.tensor.matmul(ps[:sz, :], xt[:, 128 * kk:128 * kk + sz],
                                     wT_sb[kk][:, 512:768],
                                     start=(kk == 0), stop=(kk == 15))
                nc.vector.tensor_add(v_sb[j][:sz, :], ps[:sz, :], bvB[:sz, :])
                rope_gain(j, pj)

        wtpool.release()
        ppool.release()

        # ---- phase 3: q norm-scale + transposes, then attention ----
        with tc.tile_pool(name="pat", bufs=2, space="PSUM") as pat:
            finish_ar()
            rs_q = rs_holder["rs_q"]
            rs_k_s = rs_holder["rs_k_s"]
            for j, (off, sz) in enumerate(S_TILES):
                rqs = wpool.tile([128, CH], BF, tag="rqs", bufs=2,
                                 name=f"rqs{j}")
                nc.vector.tensor_scalar_mul(rqs[:sz, :],
                                            rq_store[j][:sz, 0:CH],
                                            rs_q[:sz, j:j + 1])
                for hh in range(HPC):
                    tp = pat.tile([128, 128], BF, tag="tr")
                    nc.tensor.transpose(tp[:, :sz],
                                        rqs[:sz, D * hh:D * (hh + 1)],
                                        ident[:sz, :sz])
                    nc.vector.tensor_copy(rqT_sb[hh][:, off:off + sz],
                                          tp[:, :sz])
            rqpool.release()
            # output-projection weights: big contiguous chunks on sync+gpsimd
            # (NOT the ACT queue — its triggers would stall attention exps)
            tpool = tc.alloc_tile_pool(name="tailp", bufs=1)
            woTbig = tpool.tile([128, 16 * 2048], BF, tag="woTbig",
                                name="woTbig")
            wo_trig = [nc.sync, nc.gpsimd, nc.sync, nc.gpsimd]
            for c in range(4):
                wo_trig[c].dma_start(out=woTbig[:, 8192 * c:8192 * (c + 1)],
                                     in_=woTt[:, 8192 * c:8192 * (c + 1)])
            woT_sb = [woTbig[:, 2048 * kk:2048 * (kk + 1)] for kk in range(16)]

            att = {}

            def attn_state(hh, jc):
                st = att.get((hh, jc))
                if st is None:
                    o_ps = pat.tile([128, 880], F32, tag="o", bufs=1,
                                    name=f"o{hh}_{jc}")
                    den = wpool.tile([128, 880], BF, tag="den", bufs=2,
                                     name=f"den{hh}_{jc}")
                    pts = {}
                    st = att[(hh, jc)] = (o_ps, den, pts)
                return st

            def attn_sc(hh, jc, tlist):
                jof, jsz = SJ[jc]
                o_ps, den, pts = attn_state(hh, jc)
                for ti in tlist:
                    part, j2, toff, tsz = T_TILES[ti]
                    sc = pat.tile([128, 880], F32, tag="sc")
                    nc.tensor.matmul(
                        sc[:tsz, 0:512], kwT_sb[hh][:, toff:toff + tsz],
                        rqT_sb[hh][:, jof:jof + 512], start=True, stop=True)
                    nc.tensor.matmul(
                        sc[:tsz, 512:880], kwT_sb[hh][:, toff:toff + tsz],
                        rqT_sb[hh][:, jof + 512:jof + 880],
                        start=True, stop=True)
                    pT = wpool.tile([128, 880], BF, tag="pT", bufs=4)
                    esc = (SCALE if part == "c"
                           else rs_k_s[:tsz, j2:j2 + 1])
                    nc.scalar.activation(pT[:tsz, :], sc[:tsz, :], A.Exp,
                                         scale=esc)
                    if ti == 0:
                        nc.vector.tensor_copy(den[:, :], pT[:, :])
                    else:
                        nc.vector.tensor_add(den[:tsz, :], den[:tsz, :],
                                             pT[:tsz, :])
                    pts[ti] = pT

            def attn_pv(hh, jc, tlist):
                o_ps, den, pts = attn_state(hh, jc)
                for ti in tlist:
                    part, j2, toff, tsz = T_TILES[ti]
                    pT = pts.pop(ti)
                    vt = (cv_sb[hh][j2][:tsz, :] if part == "c"
                          else v_sb[j2][:tsz, D * hh:D * (hh + 1)])
                    last = ti == len(T_TILES) - 1
                    nc.tensor.matmul(o_ps[:, 0:512], vt, pT[:tsz, 0:512],
                                     start=(ti == 0), stop=last)
                    nc.tensor.matmul(o_ps[:, 512:880], vt, pT[:tsz, 512:880],
                                     start=(ti == 0), stop=last)

            def attn_finish(hh, jc, mid=None):
                jof, jsz = SJ[jc]
                o_ps, den, pts = att[(hh, jc)]
                # softmax denominator: PE partition-reduce, DVE approx
                # reciprocal, PE rank-1 broadcast, DVE normalize
                redps = pat.tile([128, 880], F32, tag="sc", name=f"red{hh}_{jc}")
                nc.tensor.matmul(redps[0:1, 0:512], ones_col[:, :],
                                 den[:, 0:512], start=True, stop=True)
                nc.tensor.matmul(redps[0:1, 512:880], ones_col[:, :],
                                 den[:, 512:880], start=True, stop=True)
                o_raw = wpool.tile([128, 880], BF, tag="oraw", bufs=2,
                                   name=f"oraw{hh}_{jc}")
                nc.vector.tensor_copy(o_raw[:, :jsz], o_ps[:, :jsz])
                drf = wpool.tile([1, 880], F32, tag="dln", bufs=1,
                                 name=f"dln{hh}_{jc}")
                nc.vector.reciprocal_approx_fast(drf[0:1, :], redps[0:1, :])
                denr = wpool.tile([1, 880], BF, tag="denr", bufs=2,
                                  name=f"denr{hh}_{jc}")
                nc.scalar.copy(denr[0:1, :], drf[0:1, :])
                if mid is not None:
                    mid()
                denb = pat.tile([128, 880], F32, tag="sc", name=f"denb{hh}_{jc}")
                nc.tensor.matmul(denb[:, 0:512], ones_row[:, :],
                                 denr[0:1, 0:512], start=True, stop=True)
                nc.tensor.matmul(denb[:, 512:880], ones_row[:, :],
                                 denr[0:1, 512:880], start=True, stop=True)
                nc.vector.tensor_mul(
                    oT_sb[hh][:, jof:jof + jsz], o_raw[:, :jsz], denb[:, :jsz])

            def emit_a2a(w, hh):
                nc.sync.dma_start(
                    out=a2a_in[w][hh][:].rearrange("d p s -> p d s"),
                    in_=oT_sb[hh][:, 880 * w:880 * (w + 1)]
                        .rearrange("p (d s) -> p d s", s=110))
                nc.gpsimd.collective_compute(
                    "AllToAll", mybir.AluOpType.bypass,
                    replica_groups=[core_ids],
                    ins=[a2a_in[w][hh][:]], outs=[a2a_out[w][hh][:]])

            otr_sb = {}

            def load_otr(w, hh):
                t = tpool.tile([128, 8 * 110], BF, tag=f"otr{w}_{hh}",
                               name=f"otr{w}_{hh}")
                nc.sync.dma_start(
                    out=t[:].rearrange("p (d s) -> p d s", s=110),
                    in_=a2a_out[w][hh][:].rearrange("d p s -> p d s"))
                otr_sb[(w, hh)] = t

            yp_store = {}

            def wave_y_mm(w, nlist, hhs, tags):
                for n in nlist:
                    yp = yp_store.get((w, n))
                    if yp is None:
                        yp = pat.tile([128, 512], F32, tag=tags[n],
                                      name=f"yp{w}_{n}")
                        yp_store[(w, n)] = yp
                    for kk in range(16):
                        src_c, hh = kk // 2, kk % 2
                        if hh not in hhs:
                            continue
                        nc.tensor.matmul(
                            yp[:110, :],
                            otr_sb[(w, hh)][:, 110 * src_c:110 * (src_c + 1)],
                            woT_sb[kk][:, 512 * n:512 * (n + 1)],
                            start=(kk == 0), stop=(kk == 15))

            def wave_y_chunk_out(w, n, yf):
                yp = yp_store[(w, n)]
                nc.scalar.copy(yf[:110, 512 * n:512 * (n + 1)], yp[:110, :])
                nc.sync.dma_start(
                    out=y_out[110 * w:110 * (w + 1), 512 * n:512 * (n + 1)],
                    in_=yf[:110, 512 * n:512 * (n + 1)])

            def wave_y_full(w):
                yf = wpool.tile([128, DIM], F32, tag="yf", bufs=1, name=f"yf{w}")
                for n in range(4):
                    wave_y_mm(w, [n], (0, 1), ["tr"] * 4)
                    wave_y_chunk_out(w, n, yf)

            def wave_y_out(w):
                yf = wpool.tile([128, DIM], F32, tag="yf", bufs=1, name=f"yf{w}")
                for n in range(4):
                    wave_y_chunk_out(w, n, yf)

            TRS = ["tr", "tr", "sc", "sc"]

            # chunk order: (0,0) (1,0) (0,1) (1,1); a2a emitted per
            # (wave, head) as soon as that head's wave chunk finishes
            attn_sc(0, 0, [0, 1])
            attn_pv(0, 0, [0, 1])
            for ti in range(2, 28):
                attn_sc(0, 0, [ti])
                attn_pv(0, 0, [ti])
            attn_sc(1, 0, [0, 1])
            attn_finish(0, 0)
            emit_a2a(0, 0)
            attn_pv(1, 0, [0, 1])
            for ti in range(2, 28):
                attn_sc(1, 0, [ti])
                attn_pv(1, 0, [ti])
            attn_sc(0, 1, [0, 1])
            attn_finish(1, 0)
            emit_a2a(0, 1)
            load_otr(0, 0)
            attn_pv(0, 1, [0, 1])
            for ti in range(2, 28):
                attn_sc(0, 1, [ti])
                attn_pv(0, 1, [ti])
            attn_sc(1, 1, [0, 1])
            attn_finish(0, 1)
            emit_a2a(1, 0)
            load_otr(0, 1)
            attn_pv(1, 1, [0, 1])
            for ti in range(2, 14):
                attn_sc(1, 1, [ti])
                attn_pv(1, 1, [ti])
            wave_y_full(0)
            load_otr(1, 0)
            for ti in range(14, 28):
                attn_sc(1, 1, [ti])
                attn_pv(1, 1, [ti])
            # last chunk: even-head y matmuls overlap the final AllToAll
            attn_finish(1, 1, mid=lambda: wave_y_mm(1, [0], (0,), TRS))
            emit_a2a(1, 1)
            wave_y_mm(1, [1, 2, 3], (0,), TRS)
            load_otr(1, 1)
            wave_y_mm(1, [0, 1, 2, 3], (1,), TRS)
            wave_y_out(1)
        tpool.release()


def _build():
    if "nc" not in _CACHE:
        nc = bacc.Bacc("TRN2", target_bir_lowering=False, debug=False,
                       num_devices=NCORES)
        _emit(nc)
        nc.compile()
        _CACHE["nc"] = nc
    return _CACHE["nc"]


def _make_fcomb(freqs):
    F, H, W = 2, 20, 44
    fr = np.asarray(freqs, np.float32)  # [1024, 64, 2]
    fpart = np.broadcast_to(fr[5:7, None, None, 0:22], (F, H, W, 22, 2))
    hpart = np.broadcast_to(fr[None, 0:H, None, 22:43], (F, H, W, 21, 2))
    wpart = np.broadcast_to(fr[None, None, 0:W, 43:64], (F, H, W, 21, 2))
    return np.concatenate([fpart, hpart, wpart], axis=3).reshape(S, 64, 2)


def _tile_major(a):
    """[S, C] -> [128, NT*C] tile-major (per-partition contiguous)."""
    C = a.shape[1]
    ap = np.zeros((NT * 128, C), np.float32)
    ap[:S] = a
    return np.ascontiguousarray(
        ap.reshape(NT, 128, C).transpose(1, 0, 2).reshape(128, NT * C))


def kernel(x, wq, bq, wk, bk, wv, bv, wo, bo, gq, gk, freqs, cache_k, cache_v):
    x = np.asarray(x, np.float32)
    wq, wk, wv, wo = (np.asarray(a, np.float32) for a in (wq, wk, wv, wo))
    bq, bk, bv, bo = (np.asarray(a, np.float32) for a in (bq, bk, bv, bo))
    gq, gk = np.asarray(gq, np.float32), np.asarray(gk, np.float32)
    cache_k = np.asarray(cache_k, np.float32)
    cache_v = np.asarray(cache_v, np.float32)

    fcomb = _make_fcomb(freqs)  # [S, 64, 2]
    fr_t, fi_t = fcomb[..., 0], fcomb[..., 1]  # [S, 64]
    # pre-tiled x^T: xT[j, p, kk*128+c] = x[128j+c, 128kk+p]
    xp = np.zeros((NT * 128, DIM), np.float32)
    xp[:S] = x[0]
    xT = np.ascontiguousarray(
        xp.reshape(NT, 128, 16, 128).transpose(0, 3, 2, 1).reshape(NT, 128, DIM)
    ).astype(BF16)

    # de-interleave rope channel pairs within each head: [2c] then [2c+1]
    # (applied consistently to wq/wk rows, their biases/gains, and the
    # transposed k-cache, so attention dot products are unchanged)
    perm = np.concatenate([np.arange(0, D, 2), np.arange(1, D, 2)])
    qk_perm = np.concatenate([h * D + perm for h in range(NH)])
    wqp, wkp = wq[qk_perm], wk[qk_perm]
    bqp, bkp = bq[qk_perm], bk[qk_perm]
    gqp, gkp = gq[qk_perm], gk[qk_perm]
    ck_perm = cache_k[0, WIN0:WIN0 + S][:, :, perm]  # [S, NH, D] channel-permuted

    woT_full = np.ascontiguousarray(wo.T).astype(np.float32)  # [DIM, DIM]
    in_maps = []
    for c in range(NCORES):
        hs = slice(CH * c, CH * (c + 1))
        h0 = HPC * c
        wTc = np.concatenate([wqp[hs].T, wkp[hs].T, wv[hs].T], axis=1)
        # tile-major: wTt[p, kk*768+cc] = wTc[128kk+p, cc]
        wTt = np.ascontiguousarray(
            wTc.reshape(16, 128, 768).transpose(1, 0, 2).reshape(128, 16 * 768)
        ).astype(BF16)
        woTt = np.ascontiguousarray(
            woT_full.reshape(16, 128, 2048).transpose(1, 0, 2)
            .reshape(128, 16 * 2048)).astype(BF16)
        ckTc = np.ascontiguousarray(
            ck_perm[:, h0:h0 + HPC, :].transpose(1, 2, 0)
        ).astype(BF16)  # [HPC, D, S]
        # pre-tiled cache-v: cvc[hh, p, j*128+d] = cv_window[128j+p, h, d]
        cw = np.zeros((NT * 128, HPC, D), np.float32)
        cw[:S] = cache_v[0, WIN0:WIN0 + S, h0:h0 + HPC, :]
        cvc = np.ascontiguousarray(
            cw.reshape(NT, 128, HPC, D).transpose(2, 1, 0, 3).reshape(HPC, 128, NT * D)
        ).astype(BF16)
        # rope tables with per-block gains folded in; blocks are
        # (q-h0, q-h1, k-h0, k-h1), each [64 even | 64 odd] channels
        ge = [gqp[hs][0:64], gqp[hs][128:192], gkp[hs][0:64], gkp[hs][128:192]]
        go = [gqp[hs][64:128], gqp[hs][192:256],
              gkp[hs][64:128], gkp[hs][192:256]]
        tabs = []
        for src, gl in ((fr_t, ge), (fi_t, go), (fi_t, ge), (fr_t, go)):
            tab = np.concatenate([src * gl[b][None, :] for b in range(4)],
                                 axis=1)  # [S, 256]
            tabs.append(_tile_major(tab).astype(BF16))
        gb = np.concatenate([bqp[hs], bkp[hs], bv[hs]])
        in_maps.append({
            "xT": xT, "wTt": wTt, "woTt": woTt,
            "ckT": ckTc, "cv": cvc,
            "ftab0": tabs[0], "ftab1": tabs[1], "ftab2": tabs[2],
            "ftab3": tabs[3],
            "gb": np.ascontiguousarray(gb)[None, :].astype(np.float32),
        })

    nc = _build()
    res = run_bass_kernel_spmd(nc, in_maps, list(range(NCORES)))
    _CACHE["last_result"] = res
    # all-to-all layout: core c returns rows [110c:110c+110] and
    # [880+110c:880+110c+110]
    y = np.empty((S, DIM), np.float32)
    for c in range(NCORES):
        yc = res.results[c]["y"]
        y[110 * c:110 * (c + 1)] = yc[:110]
        y[880 + 110 * c:880 + 110 * (c + 1)] = yc[110:]
    return (y + bo[None, :]).reshape(1, S, DIM).astype(np.float32)


# revision 14
# speedup vs baseline: 1.1083x; 1.0193x over previous
"""Trainium2 Bass kernel for nn_CausalWanSelfAttention (sparse_attention).

Strategy: tensor-parallel over heads across 8 NeuronCores. Each core owns
2 of the 16 heads and processes all 1760 tokens:
  - fused QKV projection (bf16 matmuls, PSUM-accumulated over 16 k-tiles)
  - RMS-norm: local sum-of-squares, ONE tiny AllReduce for the full-2048
    channel statistic, ln/exp-based rsqrt on ACT
  - rope applied BEFORE normalization (they commute): k transposes happen
    pre-AllReduce; the k-norm scale is folded into the exp's per-partition
    scale; the q-norm scale is a per-partition tensor_scalar multiply; the
    per-channel gains are folded into the host-built rope tables
  - attention in transposed layout: scores^T = kw^T(T) @ rq^T, exp on ACT,
    PV accumulation on PE; softmax denominator via DVE accumulation + PE
    ones-matmul partition reduce + DVE approx reciprocal + PE rank-1
    broadcast matmul; normalize with a DVE multiply
  - output projection, bf16 AllToAll of o^T so each core emits 220 rows;
    the final y-wave is split into even/odd-head halves so the even half
    overlaps the last AllToAll
Host side (free): input slicing/transposition/bf16 casts, rope freq
tables (per-partition contiguous), final concat + output bias.
"""
import sys

for _p in ("/opt/trn_rl_repo", "/root/.axon_site/_ro/trn_rl_repo"):
    if _p not in sys.path:
        sys.path.append(_p)

import numpy as np
import ml_dtypes

import concourse.bass as bass
import concourse.bacc as bacc
import concourse.mybir as mybir
from concourse import bass_isa
from concourse.tile import TileContext
from concourse.bass_utils import run_bass_kernel_spmd
from concourse.masks import make_identity

BF16 = ml_dtypes.bfloat16
S, DIM, NH, D = 1760, 2048, 16, 128
TW = 3520          # attention window length
WIN0 = 2640        # cache rows [2640:4400] form the first half of the window
NCORES, HPC = 8, 2
CH = HPC * D       # 256 channels per core
EPS = 1e-6
SCALE = 1.0 / float(np.sqrt(D))
S_OUT = S // NCORES  # 220 rows of output per core

S_TILES = [(i * 128, min(128, S - i * 128)) for i in range((S + 127) // 128)]
NT = len(S_TILES)  # 14
# window t-tiles: cache part [0,1760) then new part [1760,3520)
T_TILES = ([("c", j, off, sz) for j, (off, sz) in enumerate(S_TILES)]
           + [("n", j, off + S, sz) for j, (off, sz) in enumerate(S_TILES)])
SJ = [(0, 880), (880, 880)]  # attention s-chunks

_CACHE = {}


def _emit(nc):
    dt = mybir.dt
    BF, F32 = dt.bfloat16, dt.float32
    A = mybir.ActivationFunctionType
    Op = mybir.AluOpType
    core_ids = list(range(NCORES))

    xT = nc.declare_dram_parameter("xT", [NT, 128, DIM], BF, isOutput=False)
    wTt = nc.declare_dram_parameter("wTt", [128, 16 * 768], BF, isOutput=False)
    woTt = nc.declare_dram_parameter("woTt", [128, 16 * 2048], BF,
                                     isOutput=False)
    ckT = nc.declare_dram_parameter("ckT", [HPC, D, S], BF, isOutput=False)
    cv = nc.declare_dram_parameter("cv", [HPC, 128, NT * D], BF, isOutput=False)
    # rope tables with gains folded in, tile-major per-partition contiguous
    ftab = [nc.declare_dram_parameter(f"ftab{i}", [128, NT * 256], BF,
                                      isOutput=False) for i in range(4)]
    gbd = nc.declare_dram_parameter("gb", [1, 3 * CH], F32, isOutput=False)
    y_out = nc.declare_dram_parameter("y", [S_OUT, DIM], F32, isOutput=True)

    ss_in = nc.dram_tensor("ss_in", [128, 28], F32)
    ss_out = nc.dram_tensor("ss_out", [128, 28], F32, addr_space="Shared")
    # o-matrix all-to-all: two waves (s 0:880 and 880:1760) x two heads;
    # each core ends up with o^T columns for its own 110-row slice
    a2a_in = [[nc.dram_tensor(f"a2a_in{w}_{h}", [NCORES, D, 110], BF)
               for h in range(2)] for w in range(2)]
    a2a_out = [[nc.dram_tensor(f"a2a_out{w}_{h}", [NCORES, D, 110], BF)
                for h in range(2)] for w in range(2)]

    from contextlib import ExitStack
    with TileContext(nc) as tc, ExitStack() as stack:
        cpool = stack.enter_context(tc.tile_pool(name="const", bufs=1))
        wpool = stack.enter_context(tc.tile_pool(name="work", bufs=3))
        rqpool = tc.alloc_tile_pool(name="rqp", bufs=1)
        ppool = tc.alloc_tile_pool(name="projp", bufs=1)
        wtpool = tc.alloc_tile_pool(name="wtp", bufs=1)

        # ---- startup DMAs: xt0 + wT chunks first, spread across queues ----
        xt0 = wpool.tile([128, DIM], BF, tag="xt0", bufs=1, name="xt0pre")
        nc.sync.dma_start(out=xt0[:], in_=xT[0])
        gb_row = cpool.tile([1, 3 * CH], F32, tag="gb_row")
        nc.scalar.dma_start(out=gb_row[:], in_=gbd[:])
        wTbig = wtpool.tile([128, 16 * 768], BF, tag="wTbig", name="wTbig")
        wq_trig = [nc.scalar, nc.gpsimd, nc.sync, nc.scalar]
        for c in range(4):
            wq_trig[c].dma_start(out=wTbig[:, 3072 * c:3072 * (c + 1)],
                                 in_=wTt[:, 3072 * c:3072 * (c + 1)])
        wT_sb = [wTbig[:, 768 * kk:768 * (kk + 1)] for kk in range(16)]
        gb_full = cpool.tile([128, 3 * CH], F32, tag="gb_full")
        nc.gpsimd.partition_broadcast(gb_full[:, 0:2 * CH], gb_row[:, 0:2 * CH])
        nc.gpsimd.partition_broadcast(gb_full[:, 2 * CH:3 * CH],
                                      gb_row[:, 2 * CH:3 * CH])
        bqkB = gb_full[:, 0:2 * CH]
        bvB = gb_full[:, 2 * CH:3 * CH]

        ident = cpool.tile([128, 128], BF, tag="ident")
        make_identity(nc, ident[:])
        ones_col = cpool.tile([128, 1], BF, tag="ones_col")
        nc.gpsimd.memset(ones_col[:], 1.0)
        ones_row = cpool.tile([1, 128], BF, tag="ones_row")
        nc.gpsimd.memset(ones_row[:], 1.0)

        # rope tables (gains folded in): frA, fiA, fiB, frB — resident.
        # First halves now; second halves + kwT after the ss DMA so the
        # tiny AllReduce input is not stuck behind bulk transfers.
        FH = 7 * 256
        ftab_sb = []
        for i in range(4):
            t = cpool.tile([128, NT * 256], BF, tag=f"ftab{i}", name=f"ftab{i}")
            nc.scalar.dma_start(out=t[:, 0:FH], in_=ftab[i][:, 0:FH])
            ftab_sb.append(t)

        # attention constants (cache halves of the window): tiles now,
        # DMAs deferred past the ss_in DMA
        kwT_sb = []
        cv_big = []
        cv_sb = [[], []]
        for hh in range(HPC):
            t = cpool.tile([128, TW], BF, tag=f"kwT{hh}", name=f"kwT{hh}")
            kwT_sb.append(t)
            big = cpool.tile([128, NT * D], BF, tag=f"cva{hh}", name=f"cva{hh}")
            cv_big.append(big)
            cv_sb[hh] = [big[:, j * D:(j + 1) * D] for j in range(NT)]

        qk_sb, v_sb, rq_store = [], [], {}
        for j in range(NT):
            qk_sb.append(ppool.tile([128, 2 * CH], F32, tag=f"qk{j}",
                                    name=f"qk{j}"))
            v_sb.append(cpool.tile([128, CH], BF, tag=f"v{j}", name=f"v{j}"))

        rqT_sb = [cpool.tile([128, S], BF, tag=f"rqT{hh}", name=f"rqT{hh}")
                  for hh in range(HPC)]
        oT_sb = [cpool.tile([128, S], BF, tag=f"oT{hh}", name=f"oT{hh}")
                 for hh in range(HPC)]

        ss = cpool.tile([128, 28], F32, tag="ss")
        nc.gpsimd.memset(ss[:], 0.0)
        eps_ap = cpool.tile([128, 1], F32, tag="eps_ap")
        nc.gpsimd.memset(eps_ap[:], EPS)

        rs_holder = {}

        def issue_ar():
            nc.scalar.dma_start(out=ss_in[:], in_=ss[:])
            nc.gpsimd.collective_compute(
                "AllReduce", Op.add, replica_groups=[core_ids],
                ins=[ss_in[:]], outs=[ss_out[:]])
            for i in range(4):
                nc.scalar.dma_start(out=ftab_sb[i][:, FH:],
                                    in_=ftab[i][:, FH:])
            for hh in range(HPC):
                nc.scalar.dma_start(out=kwT_sb[hh][:, 0:S], in_=ckT[hh])

        def finish_ar():
            ssg = cpool.tile([128, 28], F32, tag="ssg")
            nc.sync.dma_start(out=ssg[:], in_=ss_out[:])
            tmp = wpool.tile([128, 28], F32, tag="rstmp", name="rstmp")
            nc.scalar.activation(tmp[:], ssg[:], A.Ln, scale=1.0 / DIM,
                                 bias=eps_ap[:])
            rqk = cpool.tile([128, 28], F32, tag="rqk", name="rqk")
            nc.scalar.activation(rqk[:], tmp[:], A.Exp, scale=-0.5)
            rs_holder["rs_q"] = rqk[:, 0:14]
            rsk = cpool.tile([128, 14], F32, tag="rsk", name="rsk")
            nc.vector.tensor_scalar_mul(rsk[:], rqk[:, 14:28], SCALE)
            rs_holder["rs_k_s"] = rsk

        # rope on un-normalized q and k (rope commutes with the per-row rms
        # scale); per-channel gains are folded into the ftab tables
        def rope_gain(j, ktr_pool):
            off, sz = S_TILES[j]
            fA = ftab_sb[0][:sz, 256 * j:256 * (j + 1)].rearrange(
                "p (b c) -> p b c", b=4)
            fiA = ftab_sb[1][:sz, 256 * j:256 * (j + 1)].rearrange(
                "p (b c) -> p b c", b=4)
            fiB = ftab_sb[2][:sz, 256 * j:256 * (j + 1)].rearrange(
                "p (b c) -> p b c", b=4)
            frB = ftab_sb[3][:sz, 256 * j:256 * (j + 1)].rearrange(
                "p (b c) -> p b c", b=4)
            q3 = qk_sb[j][:sz, :].rearrange("p (b c) -> p b c", b=4)
            qe, qo = q3[:, :, 0:64], q3[:, :, 64:128]
            rq4 = rqpool.tile([128, 2 * CH], BF, tag=f"rq{j}", name=f"rq{j}")
            r3 = rq4[:sz, :].rearrange("p (b c) -> p b c", b=4)
            t1 = wpool.tile([128, 256], BF, tag="ropet1")
            t2 = wpool.tile([128, 256], BF, tag="ropet2")
            t13 = t1[:sz, :].rearrange("p (b c) -> p b c", b=4)
            t23 = t2[:sz, :].rearrange("p (b c) -> p b c", b=4)
            nc.vector.tensor_mul(t13, qe, fA)
            nc.vector.tensor_mul(t23, qo, fiA)
            nc.vector.tensor_sub(r3[:, :, 0:64], t13, t23)
            t3 = wpool.tile([128, 256], BF, tag="ropet1")
            t4 = wpool.tile([128, 256], BF, tag="ropet2")
            t33 = t3[:sz, :].rearrange("p (b c) -> p b c", b=4)
            t43 = t4[:sz, :].rearrange("p (b c) -> p b c", b=4)
            nc.vector.tensor_mul(t33, qe, fiB)
            nc.vector.tensor_mul(t43, qo, frB)
            nc.vector.tensor_add(r3[:, :, 64:128], t33, t43)
            rq_store[j] = rq4
            # k transposes now (pre-AllReduce): k-norm folds into exp scale
            for hh in range(HPC):
                tp = ktr_pool.tile([128, 128], BF, tag="ktr")
                nc.tensor.transpose(tp[:, :sz],
                                    rq4[:sz, CH + D * hh:CH + D * (hh + 1)],
                                    ident[:sz, :sz])
                nc.scalar.copy(kwT_sb[hh][:, S + off:S + off + sz], tp[:, :sz])

        xt_trig = [nc.sync, nc.gpsimd]

        xt_ctr = [1]  # slot 0 is the prefetched xt0pre

        def load_xt(j):
            n = xt_ctr[0]
            xt_ctr[0] += 1
            xt = wpool.tile([128, DIM], BF, tag=f"xt{n % 3}", bufs=1,
                            name=f"xt_{n}")
            xt_trig[j % 2].dma_start(out=xt[:], in_=xT[j])
            return xt

        # ---- phase 1: q/k projection + sum-of-squares; then one AllReduce --
        with tc.tile_pool(name="pj", bufs=2, space="PSUM") as pj:
            for j, (off, sz) in enumerate(S_TILES):
                xt = xt0 if j == 0 else load_xt(j)
                ps = pj.tile([128, 512], F32, tag="qk")
                for kk in range(16):
                    nc.tensor.matmul(ps[:sz, 0:512], xt[:, 128 * kk:128 * kk + sz],
                                     wT_sb[kk][:, 0:512],
                                     start=(kk == 0), stop=(kk == 15))
                nc.vector.tensor_add(qk_sb[j][:sz, :], ps[:sz, :], bqkB[:sz, :])
                sq = wpool.tile([128, CH], F32, tag="sqscratch", bufs=2)
                nc.scalar.activation(sq[:sz, :], qk_sb[j][:sz, 0:CH], A.Square,
                                     accum_out=ss[:sz, j:j + 1])
                sq2 = wpool.tile([128, CH], F32, tag="sqscratch", bufs=2)
                nc.scalar.activation(sq2[:sz, :], qk_sb[j][:sz, CH:2 * CH],
                                     A.Square, accum_out=ss[:sz, 14 + j:15 + j])
            issue_ar()
            # ---- phase 2: v projection + rope + k transposes (AR in flight) --
            for j, (off, sz) in enumerate(S_TILES):
                xt = load_xt(j)
                ps = pj.tile([128, CH], F32, tag="v")
                for kk in range(16):
                    nc.tensor.matmul(ps[:sz, :], xt[:, 128 * kk:128 * kk + sz],
                                     wT_sb[kk][:, 512:768],
                                     start=(kk == 0), stop=(kk == 15))
                nc.vector.tensor_add(v_sb[j][:sz, :], ps[:sz, :], bvB[:sz, :])
                rope_gain(j, pj)

        wtpool.release()
        ppool.release()
        for hh in range(HPC):
            nc.sync.dma_start(out=cv_big[hh][:], in_=cv[hh])

        # ---- phase 3: q norm-scale + transposes, then attention ----
        with tc.tile_pool(name="pat", bufs=2, space="PSUM") as pat:
            finish_ar()
            rs_q = rs_holder["rs_q"]
            rs_k_s = rs_holder["rs_k_s"]
            for j, (off, sz) in enumerate(S_TILES):
                rqs = wpool.tile([128, CH], BF, tag="rqs", bufs=2,
                                 name=f"rqs{j}")
                nc.vector.tensor_scalar_mul(rqs[:sz, :],
                                            rq_store[j][:sz, 0:CH],
                                            rs_q[:sz, j:j + 1])
                for hh in range(HPC):
                    tp = pat.tile([128, 128], BF, tag="tr")
                    nc.tensor.transpose(tp[:, :sz],
                                        rqs[:sz, D * hh:D * (hh + 1)],
                                        ident[:sz, :sz])
                    nc.vector.tensor_copy(rqT_sb[hh][:, off:off + sz],
                                          tp[:, :sz])
            rqpool.release()
            # output-projection weights: big contiguous chunks on sync+gpsimd
            # (NOT the ACT queue — its triggers would stall attention exps)
            tpool = tc.alloc_tile_pool(name="tailp", bufs=1)
            woTbig = tpool.tile([128, 16 * 2048], BF, tag="woTbig",
                                name="woTbig")
            wo_trig = [nc.sync, nc.gpsimd, nc.sync, nc.gpsimd]
            for c in range(4):
                wo_trig[c].dma_start(out=woTbig[:, 8192 * c:8192 * (c + 1)],
                                     in_=woTt[:, 8192 * c:8192 * (c + 1)])
            woT_sb = [woTbig[:, 2048 * kk:2048 * (kk + 1)] for kk in range(16)]

            att = {}

            def attn_state(hh, jc):
                st = att.get((hh, jc))
                if st is None:
                    o_ps = pat.tile([128, 880], F32, tag="o", bufs=1,
                                    name=f"o{hh}_{jc}")
                    den = wpool.tile([128, 880], BF, tag="den", bufs=2,
                                     name=f"den{hh}_{jc}")
                    pts = {}
                    st = att[(hh, jc)] = (o_ps, den, pts)
                return st

            def attn_sc(hh, jc, tlist):
                jof, jsz = SJ[jc]
                o_ps, den, pts = attn_state(hh, jc)
                for ti in tlist:
                    part, j2, toff, tsz = T_TILES[ti]
                    sc = pat.tile([128, 880], F32, tag="sc")
                    nc.tensor.matmul(
                        sc[:tsz, 0:512], kwT_sb[hh][:, toff:toff + tsz],
                        rqT_sb[hh][:, jof:jof + 512], start=True, stop=True)
                    nc.tensor.matmul(
                        sc[:tsz, 512:880], kwT_sb[hh][:, toff:toff + tsz],
                        rqT_sb[hh][:, jof + 512:jof + 880],
                        start=True, stop=True)
                    pT = wpool.tile([128, 880], BF, tag="pT", bufs=4)
                    esc = (SCALE if part == "c"
                           else rs_k_s[:tsz, j2:j2 + 1])
                    nc.scalar.activation(pT[:tsz, :], sc[:tsz, :], A.Exp,
                                         scale=esc)
                    if ti == 0:
                        nc.vector.tensor_copy(den[:, :], pT[:, :])
                    else:
                        nc.vector.tensor_add(den[:tsz, :], den[:tsz, :],
                                             pT[:tsz, :])
                    pts[ti] = pT

            def attn_pv(hh, jc, tlist):
                o_ps, den, pts = attn_state(hh, jc)
                for ti in tlist:
                    part, j2, toff, tsz = T_TILES[ti]
                    pT = pts.pop(ti)
                    vt = (cv_sb[hh][j2][:tsz, :] if part == "c"
                          else v_sb[j2][:tsz, D * hh:D * (hh + 1)])
                    last = ti == len(T_TILES) - 1
                    nc.tensor.matmul(o_ps[:, 0:512], vt, pT[:tsz, 0:512],
                                     start=(ti == 0), stop=last)
                    nc.tensor.matmul(o_ps[:, 512:880], vt, pT[:tsz, 512:880],
                                     start=(ti == 0), stop=last)

            def attn_finish(hh, jc, mid=None):
                jof, jsz = SJ[jc]
                o_ps, den, pts = att[(hh, jc)]
                # softmax denominator: PE partition-reduce, DVE approx
                # reciprocal, PE rank-1 broadcast, DVE normalize
                redps = pat.tile([128, 880], F32, tag="sc", name=f"red{hh}_{jc}")
                nc.tensor.matmul(redps[0:1, 0:512], ones_col[:, :],
                                 den[:, 0:512], start=True, stop=True)
                nc.tensor.matmul(redps[0:1, 512:880], ones_col[:, :],
                                 den[:, 512:880], start=True, stop=True)
                o_raw = wpool.tile([128, 880], BF, tag="oraw", bufs=2,
                                   name=f"oraw{hh}_{jc}")
                nc.vector.tensor_copy(o_raw[:, :jsz], o_ps[:, :jsz])
                drf = wpool.tile([1, 880], F32, tag="dln", bufs=1,
                                 name=f"dln{hh}_{jc}")
                nc.vector.reciprocal_approx_fast(drf[0:1, :], redps[0:1, :])
                denr = wpool.tile([1, 880], BF, tag="denr", bufs=2,
                                  name=f"denr{hh}_{jc}")
                nc.scalar.copy(denr[0:1, :], drf[0:1, :])
                if mid is not None:
                    mid()
                denb = pat.tile([128, 880], F32, tag="sc", name=f"denb{hh}_{jc}")
                nc.tensor.matmul(denb[:, 0:512], ones_row[:, :],
                                 denr[0:1, 0:512], start=True, stop=True)
                nc.tensor.matmul(denb[:, 512:880], ones_row[:, :],
                                 denr[0:1, 512:880], start=True, stop=True)
                nc.vector.tensor_mul(
                    oT_sb[hh][:, jof:jof + jsz], o_raw[:, :jsz], denb[:, :jsz])

            def emit_a2a(w, hh):
                nc.sync.dma_start(
                    out=a2a_in[w][hh][:].rearrange("d p s -> p d s"),
                    in_=oT_sb[hh][:, 880 * w:880 * (w + 1)]
                        .rearrange("p (d s) -> p d s", s=110))
                nc.gpsimd.collective_compute(
                    "AllToAll", mybir.AluOpType.bypass,
                    replica_groups=[core_ids],
                    ins=[a2a_in[w][hh][:]], outs=[a2a_out[w][hh][:]])

            otr_sb = {}

            def load_otr(w, hh):
                t = tpool.tile([128, 8 * 110], BF, tag=f"otr{w}_{hh}",
                               name=f"otr{w}_{hh}")
                nc.sync.dma_start(
                    out=t[:].rearrange("p (d s) -> p d s", s=110),
                    in_=a2a_out[w][hh][:].rearrange("d p s -> p d s"))
                otr_sb[(w, hh)] = t

            yp_store = {}

            def wave_y_mm(w, nlist, hhs, tags):
                for n in nlist:
                    yp = yp_store.get((w, n))
                    if yp is None:
                        yp = pat.tile([128, 512], F32, tag=tags[n],
                                      name=f"yp{w}_{n}")
                        yp_store[(w, n)] = yp
                    for kk in range(16):
                        src_c, hh = kk // 2, kk % 2
                        if hh not in hhs:
                            continue
                        nc.tensor.matmul(
                            yp[:110, :],
                            otr_sb[(w, hh)][:, 110 * src_c:110 * (src_c + 1)],
                            woT_sb[kk][:, 512 * n:512 * (n + 1)],
                            start=(kk == 0), stop=(kk == 15))

            def wave_y_chunk_out(w, n, yf):
                yp = yp_store[(w, n)]
                nc.scalar.copy(yf[:110, 512 * n:512 * (n + 1)], yp[:110, :])
                nc.sync.dma_start(
                    out=y_out[110 * w:110 * (w + 1), 512 * n:512 * (n + 1)],
                    in_=yf[:110, 512 * n:512 * (n + 1)])

            def wave_y_full(w):
                yf = wpool.tile([128, DIM], F32, tag="yf", bufs=1, name=f"yf{w}")
                for n in range(4):
                    wave_y_mm(w, [n], (0, 1), ["tr"] * 4)
                    wave_y_chunk_out(w, n, yf)

            def wave_y_out(w):
                yf = wpool.tile([128, DIM], F32, tag="yf", bufs=1, name=f"yf{w}")
                for n in range(4):
                    wave_y_chunk_out(w, n, yf)

            TRS = ["tr", "tr", "sc", "sc"]

            # chunk order: (0,0) (1,0) (0,1) (1,1); a2a emitted per
            # (wave, head) as soon as that head's wave chunk finishes
            attn_sc(0, 0, [0, 1])
            attn_pv(0, 0, [0, 1])
            for ti in range(2, 28):
                attn_sc(0, 0, [ti])
                attn_pv(0, 0, [ti])
            attn_sc(1, 0, [0, 1])
            attn_finish(0, 0)
            emit_a2a(0, 0)
            attn_pv(1, 0, [0, 1])
            for ti in range(2, 28):
                attn_sc(1, 0, [ti])
                attn_pv(1, 0, [ti])
            attn_sc(0, 1, [0, 1])
            attn_finish(1, 0)
            emit_a2a(0, 1)
            load_otr(0, 0)
            attn_pv(0, 1, [0, 1])
            for ti in range(2, 28):
                attn_sc(0, 1, [ti])
                attn_pv(0, 1, [ti])
            attn_sc(1, 1, [0, 1])
            attn_finish(0, 1)
            emit_a2a(1, 0)
            load_otr(0, 1)
            attn_pv(1, 1, [0, 1])
            for ti in range(2, 14):
                attn_sc(1, 1, [ti])
                attn_pv(1, 1, [ti])
            wave_y_full(0)
            load_otr(1, 0)
            for ti in range(14, 28):
                attn_sc(1, 1, [ti])
                attn_pv(1, 1, [ti])
            # last chunk: even-head y matmuls overlap the final AllToAll
            attn_finish(1, 1, mid=lambda: wave_y_mm(1, [0], (0,), TRS))
            emit_a2a(1, 1)
            wave_y_mm(1, [1, 2, 3], (0,), TRS)
            load_otr(1, 1)
            wave_y_mm(1, [0, 1, 2, 3], (1,), TRS)
            wave_y_out(1)
        tpool.release()


def _build():
    if "nc" not in _CACHE:
        nc = bacc.Bacc("TRN2", target_bir_lowering=False, debug=False,
                       num_devices=NCORES)
        _emit(nc)
        nc.compile()
        _CACHE["nc"] = nc
    return _CACHE["nc"]


def _make_fcomb(freqs):
    F, H, W = 2, 20, 44
    fr = np.asarray(freqs, np.float32)  # [1024, 64, 2]
    fpart = np.broadcast_to(fr[5:7, None, None, 0:22], (F, H, W, 22, 2))
    hpart = np.broadcast_to(fr[None, 0:H, None, 22:43], (F, H, W, 21, 2))
    wpart = np.broadcast_to(fr[None, None, 0:W, 43:64], (F, H, W, 21, 2))
    return np.concatenate([fpart, hpart, wpart], axis=3).reshape(S, 64, 2)


def _tile_major(a):
    """[S, C] -> [128, NT*C] tile-major (per-partition contiguous)."""
    C = a.shape[1]
    ap = np.zeros((NT * 128, C), np.float32)
    ap[:S] = a
    return np.ascontiguousarray(
        ap.reshape(NT, 128, C).transpose(1, 0, 2).reshape(128, NT * C))


def kernel(x, wq, bq, wk, bk, wv, bv, wo, bo, gq, gk, freqs, cache_k, cache_v):
    x = np.asarray(x, np.float32)
    wq, wk, wv, wo = (np.asarray(a, np.float32) for a in (wq, wk, wv, wo))
    bq, bk, bv, bo = (np.asarray(a, np.float32) for a in (bq, bk, bv, bo))
    gq, gk = np.asarray(gq, np.float32), np.asarray(gk, np.float32)
    cache_k = np.asarray(cache_k, np.float32)
    cache_v = np.asarray(cache_v, np.float32)

    fcomb = _make_fcomb(freqs)  # [S, 64, 2]
    fr_t, fi_t = fcomb[..., 0], fcomb[..., 1]  # [S, 64]
    # pre-tiled x^T: xT[j, p, kk*128+c] = x[128j+c, 128kk+p]
    xp = np.zeros((NT * 128, DIM), np.float32)
    xp[:S] = x[0]
    xT = np.ascontiguousarray(
        xp.reshape(NT, 128, 16, 128).transpose(0, 3, 2, 1).reshape(NT, 128, DIM)
    ).astype(BF16)

    # de-interleave rope channel pairs within each head: [2c] then [2c+1]
    # (applied consistently to wq/wk rows, their biases/gains, and the
    # transposed k-cache, so attention dot products are unchanged)
    perm = np.concatenate([np.arange(0, D, 2), np.arange(1, D, 2)])
    qk_perm = np.concatenate([h * D + perm for h in range(NH)])
    wqp, wkp = wq[qk_perm], wk[qk_perm]
    bqp, bkp = bq[qk_perm], bk[qk_perm]
    gqp, gkp = gq[qk_perm], gk[qk_perm]
    ck_perm = cache_k[0, WIN0:WIN0 + S][:, :, perm]  # [S, NH, D] channel-permuted

    woT_full = np.ascontiguousarray(wo.T).astype(np.float32)  # [DIM, DIM]
    in_maps = []
    for c in range(NCORES):
        hs = slice(CH * c, CH * (c + 1))
        h0 = HPC * c
        wTc = np.concatenate([wqp[hs].T, wkp[hs].T, wv[hs].T], axis=1)
        # tile-major: wTt[p, kk*768+cc] = wTc[128kk+p, cc]
        wTt = np.ascontiguousarray(
            wTc.reshape(16, 128, 768).transpose(1, 0, 2).reshape(128, 16 * 768)
        ).astype(BF16)
        woTt = np.ascontiguousarray(
            woT_full.reshape(16, 128, 2048).transpose(1, 0, 2)
            .reshape(128, 16 * 2048)).astype(BF16)
        ckTc = np.ascontiguousarray(
            ck_perm[:, h0:h0 + HPC, :].transpose(1, 2, 0)
        ).astype(BF16)  # [HPC, D, S]
        # pre-tiled cache-v: cvc[hh, p, j*128+d] = cv_window[128j+p, h, d]
        cw = np.zeros((NT * 128, HPC, D), np.float32)
        cw[:S] = cache_v[0, WIN0:WIN0 + S, h0:h0 + HPC, :]
        cvc = np.ascontiguousarray(
            cw.reshape(NT, 128, HPC, D).transpose(2, 1, 0, 3).reshape(HPC, 128, NT * D)
        ).astype(BF16)
        # rope tables with per-block gains folded in; blocks are
        # (q-h0, q-h1, k-h0, k-h1), each [64 even | 64 odd] channels
        ge = [gqp[hs][0:64], gqp[hs][128:192], gkp[hs][0:64], gkp[hs][128:192]]
        go = [gqp[hs][64:128], gqp[hs][192:256],
              gkp[hs][64:128], gkp[hs][192:256]]
        tabs = []
        for src, gl in ((fr_t, ge), (fi_t, go), (fi_t, ge), (fr_t, go)):
            tab = np.concatenate([src * gl[b][None, :] for b in range(4)],
                                 axis=1)  # [S, 256]
            tabs.append(_tile_major(tab).astype(BF16))
        gb = np.concatenate([bqp[hs], bkp[hs], bv[hs]])
        in_maps.append({
            "xT": xT, "wTt": wTt, "woTt": woTt,
            "ckT": ckTc, "cv": cvc,
            "ftab0": tabs[0], "ftab1": tabs[1], "ftab2": tabs[2],
            "ftab3": tabs[3],
            "gb": np.ascontiguousarray(gb)[None, :].astype(np.float32),
        })

    nc = _build()
    res = run_bass_kernel_spmd(nc, in_maps, list(range(NCORES)))
    _CACHE["last_result"] = res
    # all-to-all layout: core c returns rows [110c:110c+110] and
    # [880+110c:880+110c+110]
    y = np.empty((S, DIM), np.float32)
    for c in range(NCORES):
        yc = res.results[c]["y"]
        y[110 * c:110 * (c + 1)] = yc[:110]
        y[880 + 110 * c:880 + 110 * (c + 1)] = yc[110:]
    return (y + bo[None, :]).reshape(1, S, DIM).astype(np.float32)


# revision 16
# speedup vs baseline: 1.1261x; 1.0160x over previous
"""Trainium2 Bass kernel for nn_CausalWanSelfAttention (sparse_attention).

Strategy: tensor-parallel over heads across 8 NeuronCores. Each core owns
2 of the 16 heads and processes all 1760 tokens:
  - fused QKV projection (bf16 matmuls, PSUM-accumulated over 16 k-tiles)
  - RMS-norm: local sum-of-squares, ONE tiny AllReduce for the full-2048
    channel statistic, ln/exp-based rsqrt on ACT
  - rope applied BEFORE normalization (they commute): k transposes happen
    pre-AllReduce; the k-norm scale is folded into the exp's per-partition
    scale; the q-norm scale is a per-partition tensor_scalar multiply; the
    per-channel gains are folded into the host-built rope tables
  - attention in transposed layout: scores^T = kw^T(T) @ rq^T, exp on ACT,
    PV accumulation on PE; softmax denominator via DVE accumulation + PE
    ones-matmul partition reduce + DVE approx reciprocal + PE rank-1
    broadcast matmul; normalize with a DVE multiply
  - output projection, bf16 AllToAll of o^T so each core emits 220 rows;
    the final y-wave is split into even/odd-head halves so the even half
    overlaps the last AllToAll
Host side (free): input slicing/transposition/bf16 casts, rope freq
tables (per-partition contiguous), final concat + output bias.
"""
import sys

for _p in ("/opt/trn_rl_repo", "/root/.axon_site/_ro/trn_rl_repo"):
    if _p not in sys.path:
        sys.path.append(_p)

import numpy as np
import ml_dtypes

import concourse.bass as bass
import concourse.bacc as bacc
import concourse.mybir as mybir
from concourse import bass_isa
from concourse.tile import TileContext
from concourse.bass_utils import run_bass_kernel_spmd
from concourse.masks import make_identity

BF16 = ml_dtypes.bfloat16
S, DIM, NH, D = 1760, 2048, 16, 128
TW = 3520          # attention window length
WIN0 = 2640        # cache rows [2640:4400] form the first half of the window
NCORES, HPC = 8, 2
CH = HPC * D       # 256 channels per core
EPS = 1e-6
SCALE = 1.0 / float(np.sqrt(D))
S_OUT = S // NCORES  # 220 rows of output per core

S_TILES = [(i * 128, min(128, S - i * 128)) for i in range((S + 127) // 128)]
NT = len(S_TILES)  # 14
# window t-tiles: cache part [0,1760) then new part [1760,3520)
T_TILES = ([("c", j, off, sz) for j, (off, sz) in enumerate(S_TILES)]
           + [("n", j, off + S, sz) for j, (off, sz) in enumerate(S_TILES)])
SJ = [(0, 880), (880, 880)]  # attention s-chunks

_CACHE = {}


def _emit(nc):
    dt = mybir.dt
    BF, F32 = dt.bfloat16, dt.float32
    A = mybir.ActivationFunctionType
    Op = mybir.AluOpType
    core_ids = list(range(NCORES))

    xT = nc.declare_dram_parameter("xT", [NT, 128, DIM], BF, isOutput=False)
    wTt = nc.declare_dram_parameter("wTt", [128, 16 * 768], BF, isOutput=False)
    woTt = nc.declare_dram_parameter("woTt", [128, 16 * 2048], BF,
                                     isOutput=False)
    ckT = nc.declare_dram_parameter("ckT", [HPC, D, S], BF, isOutput=False)
    cv = nc.declare_dram_parameter("cv", [HPC, 128, NT * D], BF, isOutput=False)
    # rope tables with gains folded in, tile-major per-partition contiguous
    ftab = [nc.declare_dram_parameter(f"ftab{i}", [128, NT * 256], BF,
                                      isOutput=False) for i in range(4)]
    gbd = nc.declare_dram_parameter("gb", [1, 3 * CH], F32, isOutput=False)
    y_out = nc.declare_dram_parameter("y", [S_OUT, DIM], F32, isOutput=True)

    ss_in = nc.dram_tensor("ss_in", [128, 28], F32)
    ss_out = nc.dram_tensor("ss_out", [128, 28], F32, addr_space="Shared")
    # o-matrix all-to-all: two waves (s 0:880 and 880:1760) x two heads;
    # each core ends up with o^T columns for its own 110-row slice
    a2a_in = [[nc.dram_tensor(f"a2a_in{w}_{h}", [NCORES, D, 110], BF)
               for h in range(2)] for w in range(2)]
    a2a_out = [[nc.dram_tensor(f"a2a_out{w}_{h}", [NCORES, D, 110], BF)
                for h in range(2)] for w in range(2)]

    from contextlib import ExitStack
    with TileContext(nc) as tc, ExitStack() as stack:
        cpool = stack.enter_context(tc.tile_pool(name="const", bufs=1))
        wpool = stack.enter_context(tc.tile_pool(name="work", bufs=3))
        rqpool = tc.alloc_tile_pool(name="rqp", bufs=1)
        ppool = tc.alloc_tile_pool(name="projp", bufs=1)
        wtpool = tc.alloc_tile_pool(name="wtp", bufs=1)

        # ---- startup DMAs: xt0 + wT chunks first, spread across queues ----
        xt0 = wpool.tile([128, DIM], BF, tag="xt0", bufs=1, name="xt0pre")
        nc.sync.dma_start(out=xt0[:], in_=xT[0])
        gb_row = cpool.tile([1, 3 * CH], F32, tag="gb_row")
        nc.scalar.dma_start(out=gb_row[:], in_=gbd[:])
        wTbig = wtpool.tile([128, 16 * 768], BF, tag="wTbig", name="wTbig")
        wq_trig = [nc.scalar, nc.gpsimd, nc.sync, nc.scalar]
        for c in range(4):
            wq_trig[c].dma_start(out=wTbig[:, 3072 * c:3072 * (c + 1)],
                                 in_=wTt[:, 3072 * c:3072 * (c + 1)])
        wT_sb = [wTbig[:, 768 * kk:768 * (kk + 1)] for kk in range(16)]
        gb_full = cpool.tile([128, 3 * CH], F32, tag="gb_full")
        nc.gpsimd.partition_broadcast(gb_full[:, 0:2 * CH], gb_row[:, 0:2 * CH])
        nc.gpsimd.partition_broadcast(gb_full[:, 2 * CH:3 * CH],
                                      gb_row[:, 2 * CH:3 * CH])
        bqkB = gb_full[:, 0:2 * CH]
        bvB = gb_full[:, 2 * CH:3 * CH]

        ident = cpool.tile([128, 128], BF, tag="ident")
        make_identity(nc, ident[:])
        ones_col = cpool.tile([128, 1], BF, tag="ones_col")
        nc.gpsimd.memset(ones_col[:], 1.0)
        ones_row = cpool.tile([1, 128], BF, tag="ones_row")
        nc.gpsimd.memset(ones_row[:], 1.0)

        # rope tables (gains folded in): frA, fiA, fiB, frB — resident.
        # First halves now; second halves + kwT after the ss DMA so the
        # tiny AllReduce input is not stuck behind bulk transfers.
        FH = 7 * 256
        ftab_sb = []
        for i in range(4):
            t = cpool.tile([128, NT * 256], BF, tag=f"ftab{i}", name=f"ftab{i}")
            nc.scalar.dma_start(out=t[:, 0:FH], in_=ftab[i][:, 0:FH])
            ftab_sb.append(t)

        # attention constants (cache halves of the window): tiles now,
        # DMAs deferred past the ss_in DMA
        kwT_sb = []
        cv_big = []
        cv_sb = [[], []]
        for hh in range(HPC):
            t = cpool.tile([128, TW], BF, tag=f"kwT{hh}", name=f"kwT{hh}")
            kwT_sb.append(t)
            big = cpool.tile([128, NT * D], BF, tag=f"cva{hh}", name=f"cva{hh}")
            cv_big.append(big)
            cv_sb[hh] = [big[:, j * D:(j + 1) * D] for j in range(NT)]

        qk_sb, v_sb, rq_store = [], [], {}
        for j in range(NT):
            qk_sb.append(ppool.tile([128, 2 * CH], F32, tag=f"qk{j}",
                                    name=f"qk{j}"))
            v_sb.append(cpool.tile([128, CH], BF, tag=f"v{j}", name=f"v{j}"))

        rqT_sb = [cpool.tile([128, S], BF, tag=f"rqT{hh}", name=f"rqT{hh}")
                  for hh in range(HPC)]
        oT_sb = [cpool.tile([128, S], BF, tag=f"oT{hh}", name=f"oT{hh}")
                 for hh in range(HPC)]

        ss = cpool.tile([128, 28], F32, tag="ss")
        nc.gpsimd.memset(ss[:], 0.0)
        eps_ap = cpool.tile([128, 1], F32, tag="eps_ap")
        nc.gpsimd.memset(eps_ap[:], EPS)

        rs_holder = {}

        def issue_ar():
            nc.scalar.dma_start(out=ss_in[:], in_=ss[:])
            nc.gpsimd.collective_compute(
                "AllReduce", Op.add, replica_groups=[core_ids],
                ins=[ss_in[:]], outs=[ss_out[:]])
            for i in range(4):
                nc.scalar.dma_start(out=ftab_sb[i][:, FH:],
                                    in_=ftab[i][:, FH:])
            for hh in range(HPC):
                nc.scalar.dma_start(out=kwT_sb[hh][:, 0:S], in_=ckT[hh])

        def finish_ar():
            ssg = cpool.tile([128, 28], F32, tag="ssg")
            nc.sync.dma_start(out=ssg[:], in_=ss_out[:])
            tmp = wpool.tile([128, 28], F32, tag="rstmp", name="rstmp")
            nc.scalar.activation(tmp[:], ssg[:], A.Ln, scale=1.0 / DIM,
                                 bias=eps_ap[:])
            rqk = cpool.tile([128, 28], F32, tag="rqk", name="rqk")
            nc.scalar.activation(rqk[:], tmp[:], A.Exp, scale=-0.5)
            rs_holder["rs_q"] = rqk[:, 0:14]
            rsk = cpool.tile([128, 14], F32, tag="rsk", name="rsk")
            nc.vector.tensor_scalar_mul(rsk[:], rqk[:, 14:28], SCALE)
            rs_holder["rs_k_s"] = rsk

        # rope on un-normalized q and k (rope commutes with the per-row rms
        # scale); per-channel gains are folded into the ftab tables
        def rope_gain(j, ktr_pool):
            off, sz = S_TILES[j]
            fA = ftab_sb[0][:sz, 256 * j:256 * (j + 1)].rearrange(
                "p (b c) -> p b c", b=4)
            fiA = ftab_sb[1][:sz, 256 * j:256 * (j + 1)].rearrange(
                "p (b c) -> p b c", b=4)
            fiB = ftab_sb[2][:sz, 256 * j:256 * (j + 1)].rearrange(
                "p (b c) -> p b c", b=4)
            frB = ftab_sb[3][:sz, 256 * j:256 * (j + 1)].rearrange(
                "p (b c) -> p b c", b=4)
            q3 = qk_sb[j][:sz, :].rearrange("p (b c) -> p b c", b=4)
            qe, qo = q3[:, :, 0:64], q3[:, :, 64:128]
            rq4 = rqpool.tile([128, 2 * CH], BF, tag=f"rq{j}", name=f"rq{j}")
            r3 = rq4[:sz, :].rearrange("p (b c) -> p b c", b=4)
            t1 = wpool.tile([128, 256], BF, tag="ropet1")
            t2 = wpool.tile([128, 256], BF, tag="ropet2")
            t13 = t1[:sz, :].rearrange("p (b c) -> p b c", b=4)
            t23 = t2[:sz, :].rearrange("p (b c) -> p b c", b=4)
            nc.vector.tensor_mul(t13, qe, fA)
            nc.vector.tensor_mul(t23, qo, fiA)
            nc.vector.tensor_sub(r3[:, :, 0:64], t13, t23)
            t3 = wpool.tile([128, 256], BF, tag="ropet1")
            t4 = wpool.tile([128, 256], BF, tag="ropet2")
            t33 = t3[:sz, :].rearrange("p (b c) -> p b c", b=4)
            t43 = t4[:sz, :].rearrange("p (b c) -> p b c", b=4)
            nc.vector.tensor_mul(t33, qe, fiB)
            nc.vector.tensor_mul(t43, qo, frB)
            nc.vector.tensor_add(r3[:, :, 64:128], t33, t43)
            rq_store[j] = rq4
            # k transposes now (pre-AllReduce): k-norm folds into exp scale
            for hh in range(HPC):
                tp = ktr_pool.tile([128, 128], BF, tag="ktr")
                nc.tensor.transpose(tp[:, :sz],
                                    rq4[:sz, CH + D * hh:CH + D * (hh + 1)],
                                    ident[:sz, :sz])
                nc.vector.tensor_copy(kwT_sb[hh][:, S + off:S + off + sz], tp[:, :sz])

        xt_trig = [nc.sync, nc.gpsimd]

        xt_ctr = [1]  # slot 0 is the prefetched xt0pre
        xt_keep = {}

        def load_xt(j, eng=None):
            n = xt_ctr[0]
            xt_ctr[0] += 1
            xt = wpool.tile([128, DIM], BF, tag=f"xt{n % 3}", bufs=1,
                            name=f"xt_{n}")
            (eng or xt_trig[j % 2]).dma_start(out=xt[:], in_=xT[j])
            return xt

        # ---- phase 1: q/k projection + sum-of-squares; then one AllReduce --
        with tc.tile_pool(name="pj", bufs=2, space="PSUM") as pj:
            for j, (off, sz) in enumerate(S_TILES):
                xt = xt0 if j == 0 else load_xt(
                    j, nc.scalar if j in (9, 11, 13) else None)
                if j >= 11:
                    xt_keep[j] = xt
                ps = pj.tile([128, 512], F32, tag="qk")
                for kk in range(16):
                    nc.tensor.matmul(ps[:sz, 0:512], xt[:, 128 * kk:128 * kk + sz],
                                     wT_sb[kk][:, 0:512],
                                     start=(kk == 0), stop=(kk == 15))
                nc.vector.tensor_add(qk_sb[j][:sz, :], ps[:sz, :], bqkB[:sz, :])
                sq = wpool.tile([128, CH], F32, tag="sqscratch", bufs=2)
                nc.scalar.activation(sq[:sz, :], qk_sb[j][:sz, 0:CH], A.Square,
                                     accum_out=ss[:sz, j:j + 1])
                sq2 = wpool.tile([128, CH], F32, tag="sqscratch", bufs=2)
                nc.scalar.activation(sq2[:sz, :], qk_sb[j][:sz, CH:2 * CH],
                                     A.Square, accum_out=ss[:sz, 14 + j:15 + j])
            issue_ar()
            # ---- phase 2: v projection + rope + k transposes (AR in flight);
            # runs j=13..0 so the last three phase-1 xt tiles are reused
            for j in list(range(NT - 1, -1, -1)):
                off, sz = S_TILES[j]
                xt = xt_keep[j] if j >= 11 else load_xt(j)
                ps = pj.tile([128, CH], F32, tag="v")
                for kk in range(16):
                    nc.tensor.matmul(ps[:sz, :], xt[:, 128 * kk:128 * kk + sz],
                                     wT_sb[kk][:, 512:768],
                                     start=(kk == 0), stop=(kk == 15))
                nc.vector.tensor_add(v_sb[j][:sz, :], ps[:sz, :], bvB[:sz, :])
                rope_gain(j, pj)

        wtpool.release()
        ppool.release()
        for hh in range(HPC):
            nc.sync.dma_start(out=cv_big[hh][:], in_=cv[hh])

        # ---- phase 3: q norm-scale + transposes, then attention ----
        with tc.tile_pool(name="pat", bufs=2, space="PSUM") as pat:
            finish_ar()
            rs_q = rs_holder["rs_q"]
            rs_k_s = rs_holder["rs_k_s"]
            for j, (off, sz) in enumerate(S_TILES):
                rqs = wpool.tile([128, CH], BF, tag="rqs", bufs=2,
                                 name=f"rqs{j}")
                nc.vector.tensor_scalar_mul(rqs[:sz, :],
                                            rq_store[j][:sz, 0:CH],
                                            rs_q[:sz, j:j + 1])
                for hh in range(HPC):
                    tp = pat.tile([128, 128], BF, tag="tr")
                    nc.tensor.transpose(tp[:, :sz],
                                        rqs[:sz, D * hh:D * (hh + 1)],
                                        ident[:sz, :sz])
                    nc.vector.tensor_copy(rqT_sb[hh][:, off:off + sz],
                                          tp[:, :sz])
            rqpool.release()
            # output-projection weights: big contiguous chunks on sync+gpsimd
            # (NOT the ACT queue — its triggers would stall attention exps)
            tpool = tc.alloc_tile_pool(name="tailp", bufs=1)
            woTbig = tpool.tile([128, 16 * 2048], BF, tag="woTbig",
                                name="woTbig")
            def load_woT(c):
                nc.gpsimd.dma_start(out=woTbig[:, 8192 * c:8192 * (c + 1)],
                                    in_=woTt[:, 8192 * c:8192 * (c + 1)])

            load_woT(0)
            load_woT(1)
            woT_sb = [woTbig[:, 2048 * kk:2048 * (kk + 1)] for kk in range(16)]

            att = {}

            def attn_state(hh, jc):
                st = att.get((hh, jc))
                if st is None:
                    o_ps = pat.tile([128, 880], F32, tag="o", bufs=1,
                                    name=f"o{hh}_{jc}")
                    den = wpool.tile([128, 880], BF, tag="den", bufs=2,
                                     name=f"den{hh}_{jc}")
                    pts = {}
                    st = att[(hh, jc)] = (o_ps, den, pts)
                return st

            def attn_sc(hh, jc, tlist):
                jof, jsz = SJ[jc]
                o_ps, den, pts = attn_state(hh, jc)
                for ti in tlist:
                    part, j2, toff, tsz = T_TILES[ti]
                    sc = pat.tile([128, 880], F32, tag="sc")
                    nc.tensor.matmul(
                        sc[:tsz, 0:512], kwT_sb[hh][:, toff:toff + tsz],
                        rqT_sb[hh][:, jof:jof + 512], start=True, stop=True)
                    nc.tensor.matmul(
                        sc[:tsz, 512:880], kwT_sb[hh][:, toff:toff + tsz],
                        rqT_sb[hh][:, jof + 512:jof + 880],
                        start=True, stop=True)
                    pT = wpool.tile([128, 880], BF, tag="pT", bufs=4)
                    esc = (SCALE if part == "c"
                           else rs_k_s[:tsz, j2:j2 + 1])
                    nc.scalar.activation(pT[:tsz, :], sc[:tsz, :], A.Exp,
                                         scale=esc)
                    if ti == 0:
                        nc.vector.tensor_copy(den[:, :], pT[:, :])
                    else:
                        nc.vector.tensor_add(den[:tsz, :], den[:tsz, :],
                                             pT[:tsz, :])
                    pts[ti] = pT

            def attn_pv(hh, jc, tlist):
                o_ps, den, pts = attn_state(hh, jc)
                for ti in tlist:
                    part, j2, toff, tsz = T_TILES[ti]
                    pT = pts.pop(ti)
                    vt = (cv_sb[hh][j2][:tsz, :] if part == "c"
                          else v_sb[j2][:tsz, D * hh:D * (hh + 1)])
                    last = ti == len(T_TILES) - 1
                    nc.tensor.matmul(o_ps[:, 0:512], vt, pT[:tsz, 0:512],
                                     start=(ti == 0), stop=last)
                    nc.tensor.matmul(o_ps[:, 512:880], vt, pT[:tsz, 512:880],
                                     start=(ti == 0), stop=last)

            def attn_finish(hh, jc, mid=None):
                jof, jsz = SJ[jc]
                o_ps, den, pts = att[(hh, jc)]
                # softmax denominator: PE partition-reduce, DVE approx
                # reciprocal, PE rank-1 broadcast, DVE normalize
                redps = pat.tile([128, 880], F32, tag="sc", name=f"red{hh}_{jc}")
                nc.tensor.matmul(redps[0:1, 0:512], ones_col[:, :],
                                 den[:, 0:512], start=True, stop=True)
                nc.tensor.matmul(redps[0:1, 512:880], ones_col[:, :],
                                 den[:, 512:880], start=True, stop=True)
                o_raw = wpool.tile([128, 880], BF, tag="oraw", bufs=2,
                                   name=f"oraw{hh}_{jc}")
                nc.vector.tensor_copy(o_raw[:, :jsz], o_ps[:, :jsz])
                drf = wpool.tile([1, 880], F32, tag="dln", bufs=1,
                                 name=f"dln{hh}_{jc}")
                nc.vector.reciprocal_approx_fast(drf[0:1, :], redps[0:1, :])
                denr = wpool.tile([1, 880], BF, tag="denr", bufs=2,
                                  name=f"denr{hh}_{jc}")
                nc.scalar.copy(denr[0:1, :], drf[0:1, :])
                if mid is not None:
                    mid()
                denb = pat.tile([128, 880], F32, tag="sc", name=f"denb{hh}_{jc}")
                nc.tensor.matmul(denb[:, 0:512], ones_row[:, :],
                                 denr[0:1, 0:512], start=True, stop=True)
                nc.tensor.matmul(denb[:, 512:880], ones_row[:, :],
                                 denr[0:1, 512:880], start=True, stop=True)
                nc.vector.tensor_mul(
                    oT_sb[hh][:, jof:jof + jsz], o_raw[:, :jsz], denb[:, :jsz])

            def emit_a2a(w, hh):
                nc.sync.dma_start(
                    out=a2a_in[w][hh][:].rearrange("d p s -> p d s"),
                    in_=oT_sb[hh][:, 880 * w:880 * (w + 1)]
                        .rearrange("p (d s) -> p d s", s=110))
                nc.gpsimd.collective_compute(
                    "AllToAll", mybir.AluOpType.bypass,
                    replica_groups=[core_ids],
                    ins=[a2a_in[w][hh][:]], outs=[a2a_out[w][hh][:]])

            otr_sb = {}

            def load_otr(w, hh):
                t = tpool.tile([128, 8 * 110], BF, tag=f"otr{w}_{hh}",
                               name=f"otr{w}_{hh}")
                nc.sync.dma_start(
                    out=t[:].rearrange("p (d s) -> p d s", s=110),
                    in_=a2a_out[w][hh][:].rearrange("d p s -> p d s"))
                otr_sb[(w, hh)] = t

            yp_store = {}

            def wave_y_mm(w, nlist, hhs, tags):
                for n in nlist:
                    yp = yp_store.get((w, n))
                    if yp is None:
                        yp = pat.tile([128, 512], F32, tag=tags[n],
                                      name=f"yp{w}_{n}")
                        yp_store[(w, n)] = yp
                    for kk in range(16):
                        src_c, hh = kk // 2, kk % 2
                        if hh not in hhs:
                            continue
                        nc.tensor.matmul(
                            yp[:110, :],
                            otr_sb[(w, hh)][:, 110 * src_c:110 * (src_c + 1)],
                            woT_sb[kk][:, 512 * n:512 * (n + 1)],
                            start=(kk == 0), stop=(kk == 15))

            def wave_y_chunk_out(w, n, yf):
                yp = yp_store[(w, n)]
                nc.scalar.copy(yf[:110, 512 * n:512 * (n + 1)], yp[:110, :])
                nc.sync.dma_start(
                    out=y_out[110 * w:110 * (w + 1), 512 * n:512 * (n + 1)],
                    in_=yf[:110, 512 * n:512 * (n + 1)])

            def wave_y_full(w):
                yf = wpool.tile([128, DIM], F32, tag="yf", bufs=1, name=f"yf{w}")
                for n in range(4):
                    wave_y_mm(w, [n], (0, 1), ["tr"] * 4)
                    wave_y_chunk_out(w, n, yf)

            def wave_y_out(w):
                yf = wpool.tile([128, DIM], F32, tag="yf", bufs=1, name=f"yf{w}")
                for n in range(4):
                    wave_y_chunk_out(w, n, yf)

            TRS = ["tr", "tr", "sc", "sc"]

            # chunk order: (0,0) (1,0) (0,1) (1,1); a2a emitted per
            # (wave, head) as soon as that head's wave chunk finishes
            attn_sc(0, 0, [0, 1])
            attn_pv(0, 0, [0, 1])
            for ti in range(2, 28):
                attn_sc(0, 0, [ti])
                attn_pv(0, 0, [ti])
            attn_sc(1, 0, [0, 1])
            attn_finish(0, 0)
            emit_a2a(0, 0)
            load_woT(2)
            attn_pv(1, 0, [0, 1])
            for ti in range(2, 28):
                attn_sc(1, 0, [ti])
                attn_pv(1, 0, [ti])
            attn_sc(0, 1, [0, 1])
            attn_finish(1, 0)
            emit_a2a(0, 1)
            load_woT(3)
            load_otr(0, 0)
            attn_pv(0, 1, [0, 1])
            for ti in range(2, 28):
                attn_sc(0, 1, [ti])
                attn_pv(0, 1, [ti])
            attn_sc(1, 1, [0, 1])
            attn_finish(0, 1)
            emit_a2a(1, 0)
            load_otr(0, 1)
            attn_pv(1, 1, [0, 1])
            for ti in range(2, 14):
                attn_sc(1, 1, [ti])
                attn_pv(1, 1, [ti])
            wave_y_full(0)
            load_otr(1, 0)
            for ti in range(14, 28):
                attn_sc(1, 1, [ti])
                attn_pv(1, 1, [ti])
            # last chunk: even-head y matmuls overlap the final AllToAll
            attn_finish(1, 1, mid=lambda: wave_y_mm(1, [0], (0,), TRS))
            emit_a2a(1, 1)
            wave_y_mm(1, [1, 2, 3], (0,), TRS)
            load_otr(1, 1)
            wave_y_mm(1, [0, 1, 2, 3], (1,), TRS)
            wave_y_out(1)
        tpool.release()


def _build():
    if "nc" not in _CACHE:
        nc = bacc.Bacc("TRN2", target_bir_lowering=False, debug=False,
                       num_devices=NCORES)
        _emit(nc)
        nc.compile()
        _CACHE["nc"] = nc
    return _CACHE["nc"]


def _make_fcomb(freqs):
    F, H, W = 2, 20, 44
    fr = np.asarray(freqs, np.float32)  # [1024, 64, 2]
    fpart = np.broadcast_to(fr[5:7, None, None, 0:22], (F, H, W, 22, 2))
    hpart = np.broadcast_to(fr[None, 0:H, None, 22:43], (F, H, W, 21, 2))
    wpart = np.broadcast_to(fr[None, None, 0:W, 43:64], (F, H, W, 21, 2))
    return np.concatenate([fpart, hpart, wpart], axis=3).reshape(S, 64, 2)


def _tile_major(a):
    """[S, C] -> [128, NT*C] tile-major (per-partition contiguous)."""
    C = a.shape[1]
    ap = np.zeros((NT * 128, C), np.float32)
    ap[:S] = a
    return np.ascontiguousarray(
        ap.reshape(NT, 128, C).transpose(1, 0, 2).reshape(128, NT * C))


def kernel(x, wq, bq, wk, bk, wv, bv, wo, bo, gq, gk, freqs, cache_k, cache_v):
    x = np.asarray(x, np.float32)
    wq, wk, wv, wo = (np.asarray(a, np.float32) for a in (wq, wk, wv, wo))
    bq, bk, bv, bo = (np.asarray(a, np.float32) for a in (bq, bk, bv, bo))
    gq, gk = np.asarray(gq, np.float32), np.asarray(gk, np.float32)
    cache_k = np.asarray(cache_k, np.float32)
    cache_v = np.asarray(cache_v, np.float32)

    fcomb = _make_fcomb(freqs)  # [S, 64, 2]
    fr_t, fi_t = fcomb[..., 0], fcomb[..., 1]  # [S, 64]
    # pre-tiled x^T: xT[j, p, kk*128+c] = x[128j+c, 128kk+p]
    xp = np.zeros((NT * 128, DIM), np.float32)
    xp[:S] = x[0]
    xT = np.ascontiguousarray(
        xp.reshape(NT, 128, 16, 128).transpose(0, 3, 2, 1).reshape(NT, 128, DIM)
    ).astype(BF16)

    # de-interleave rope channel pairs within each head: [2c] then [2c+1]
    # (applied consistently to wq/wk rows, their biases/gains, and the
    # transposed k-cache, so attention dot products are unchanged)
    perm = np.concatenate([np.arange(0, D, 2), np.arange(1, D, 2)])
    qk_perm = np.concatenate([h * D + perm for h in range(NH)])
    wqp, wkp = wq[qk_perm], wk[qk_perm]
    bqp, bkp = bq[qk_perm], bk[qk_perm]
    gqp, gkp = gq[qk_perm], gk[qk_perm]
    ck_perm = cache_k[0, WIN0:WIN0 + S][:, :, perm]  # [S, NH, D] channel-permuted

    woT_full = np.ascontiguousarray(wo.T).astype(np.float32)  # [DIM, DIM]
    in_maps = []
    for c in range(NCORES):
        hs = slice(CH * c, CH * (c + 1))
        h0 = HPC * c
        wTc = np.concatenate([wqp[hs].T, wkp[hs].T, wv[hs].T], axis=1)
        # tile-major: wTt[p, kk*768+cc] = wTc[128kk+p, cc]
        wTt = np.ascontiguousarray(
            wTc.reshape(16, 128, 768).transpose(1, 0, 2).reshape(128, 16 * 768)
        ).astype(BF16)
        woTt = np.ascontiguousarray(
            woT_full.reshape(16, 128, 2048).transpose(1, 0, 2)
            .reshape(128, 16 * 2048)).astype(BF16)
        ckTc = np.ascontiguousarray(
            ck_perm[:, h0:h0 + HPC, :].transpose(1, 2, 0)
        ).astype(BF16)  # [HPC, D, S]
        # pre-tiled cache-v: cvc[hh, p, j*128+d] = cv_window[128j+p, h, d]
        cw = np.zeros((NT * 128, HPC, D), np.float32)
        cw[:S] = cache_v[0, WIN0:WIN0 + S, h0:h0 + HPC, :]
        cvc = np.ascontiguousarray(
            cw.reshape(NT, 128, HPC, D).transpose(2, 1, 0, 3).reshape(HPC, 128, NT * D)
        ).astype(BF16)
        # rope tables with per-block gains folded in; blocks are
        # (q-h0, q-h1, k-h0, k-h1), each [64 even | 64 odd] channels
        ge = [gqp[hs][0:64], gqp[hs][128:192], gkp[hs][0:64], gkp[hs][128:192]]
        go = [gqp[hs][64:128], gqp[hs][192:256],
              gkp[hs][64:128], gkp[hs][192:256]]
        tabs = []
        for src, gl in ((fr_t, ge), (fi_t, go), (fi_t, ge), (fr_t, go)):
            tab = np.concatenate([src * gl[b][None, :] for b in range(4)],
                                 axis=1)  # [S, 256]
            tabs.append(_tile_major(tab).astype(BF16))
        gb = np.concatenate([bqp[hs], bkp[hs], bv[hs]])
        in_maps.append({
            "xT": xT, "wTt": wTt, "woTt": woTt,
            "ckT": ckTc, "cv": cvc,
            "ftab0": tabs[0], "ftab1": tabs[1], "ftab2": tabs[2],
            "ftab3": tabs[3],
            "gb": np.ascontiguousarray(gb)[None, :].astype(np.float32),
        })

    nc = _build()
    res = run_bass_kernel_spmd(nc, in_maps, list(range(NCORES)))
    _CACHE["last_result"] = res
    # all-to-all layout: core c returns rows [110c:110c+110] and
    # [880+110c:880+110c+110]
    y = np.empty((S, DIM), np.float32)
    for c in range(NCORES):
        yc = res.results[c]["y"]
        y[110 * c:110 * (c + 1)] = yc[:110]
        y[880 + 110 * c:880 + 110 * (c + 1)] = yc[110:]
    return (y + bo[None, :]).reshape(1, S, DIM).astype(np.float32)


# revision 17
# speedup vs baseline: 1.1423x; 1.0143x over previous
"""Trainium2 Bass kernel for nn_CausalWanSelfAttention (sparse_attention).

Strategy: tensor-parallel over heads across 8 NeuronCores. Each core owns
2 of the 16 heads and processes all 1760 tokens:
  - fused QKV projection (bf16 matmuls, PSUM-accumulated over 16 k-tiles)
  - RMS-norm: local sum-of-squares, ONE tiny AllReduce for the full-2048
    channel statistic, ln/exp-based rsqrt on ACT
  - rope applied BEFORE normalization (they commute): k transposes happen
    pre-AllReduce; the k-norm scale is folded into the exp's per-partition
    scale; the q-norm scale is a per-partition tensor_scalar multiply; the
    per-channel gains are folded into the host-built rope tables
  - attention in transposed layout: scores^T = kw^T(T) @ rq^T, exp on ACT,
    PV accumulation on PE; softmax denominator via DVE accumulation + PE
    ones-matmul partition reduce + DVE approx reciprocal + PE rank-1
    broadcast matmul; normalize with a DVE multiply
  - output projection, bf16 AllToAll of o^T so each core emits 220 rows;
    the final y-wave is split into even/odd-head halves so the even half
    overlaps the last AllToAll
Host side (free): input slicing/transposition/bf16 casts, rope freq
tables (per-partition contiguous), final concat + output bias.
"""
import sys

for _p in ("/opt/trn_rl_repo", "/root/.axon_site/_ro/trn_rl_repo"):
    if _p not in sys.path:
        sys.path.append(_p)

import numpy as np
import ml_dtypes

import concourse.bass as bass
import concourse.bacc as bacc
import concourse.mybir as mybir
from concourse import bass_isa
from concourse.tile import TileContext
from concourse.bass_utils import run_bass_kernel_spmd
from concourse.masks import make_identity

BF16 = ml_dtypes.bfloat16
S, DIM, NH, D = 1760, 2048, 16, 128
TW = 3520          # attention window length
WIN0 = 2640        # cache rows [2640:4400] form the first half of the window
NCORES, HPC = 8, 2
CH = HPC * D       # 256 channels per core
EPS = 1e-6
SCALE = 1.0 / float(np.sqrt(D))
S_OUT = S // NCORES  # 220 rows of output per core

S_TILES = [(i * 128, min(128, S - i * 128)) for i in range((S + 127) // 128)]
NT = len(S_TILES)  # 14
# window t-tiles: cache part [0,1760) then new part [1760,3520)
T_TILES = ([("c", j, off, sz) for j, (off, sz) in enumerate(S_TILES)]
           + [("n", j, off + S, sz) for j, (off, sz) in enumerate(S_TILES)])
SJ = [(0, 880), (880, 880)]  # attention s-chunks

_CACHE = {}


def _emit(nc):
    dt = mybir.dt
    BF, F32 = dt.bfloat16, dt.float32
    A = mybir.ActivationFunctionType
    Op = mybir.AluOpType
    core_ids = list(range(NCORES))

    xT = nc.declare_dram_parameter("xT", [NT, 128, DIM], BF, isOutput=False)
    wTt = nc.declare_dram_parameter("wTt", [128, 16 * 768], BF, isOutput=False)
    woTt = nc.declare_dram_parameter("woTt", [128, 16 * 2048], BF,
                                     isOutput=False)
    ckT = nc.declare_dram_parameter("ckT", [HPC, D, S], BF, isOutput=False)
    cv = nc.declare_dram_parameter("cv", [HPC, 128, NT * D], BF, isOutput=False)
    # rope tables with gains folded in, tile-major per-partition contiguous
    ftab = [nc.declare_dram_parameter(f"ftab{i}", [128, NT * 256], BF,
                                      isOutput=False) for i in range(4)]
    gbd = nc.declare_dram_parameter("gb", [1, 3 * CH], F32, isOutput=False)
    y_out = nc.declare_dram_parameter("y", [S_OUT, DIM], F32, isOutput=True)

    ss_in = nc.dram_tensor("ss_in", [128, 28], F32)
    ss_out = nc.dram_tensor("ss_out", [128, 28], F32, addr_space="Shared")
    # o-matrix all-to-all: two waves (s 0:880 and 880:1760) x two heads;
    # each core ends up with o^T columns for its own 110-row slice
    a2a_in = [[nc.dram_tensor(f"a2a_in{w}_{h}", [NCORES, D, 110], BF)
               for h in range(2)] for w in range(2)]
    a2a_out = [[nc.dram_tensor(f"a2a_out{w}_{h}", [NCORES, D, 110], BF)
                for h in range(2)] for w in range(2)]

    from contextlib import ExitStack
    with TileContext(nc) as tc, ExitStack() as stack:
        cpool = stack.enter_context(tc.tile_pool(name="const", bufs=1))
        wpool = stack.enter_context(tc.tile_pool(name="work", bufs=3))
        rqpool = tc.alloc_tile_pool(name="rqp", bufs=1)
        ppool = tc.alloc_tile_pool(name="projp", bufs=1)
        wtpool = tc.alloc_tile_pool(name="wtp", bufs=1)

        # ---- startup DMAs: xt0 + wT chunks first, spread across queues ----
        xt0 = wpool.tile([128, DIM], BF, tag="xt0", bufs=1, name="xt0pre")
        nc.sync.dma_start(out=xt0[:], in_=xT[0])
        gb_row = cpool.tile([1, 3 * CH], F32, tag="gb_row")
        nc.scalar.dma_start(out=gb_row[:], in_=gbd[:])
        wTbig = wtpool.tile([128, 16 * 768], BF, tag="wTbig", name="wTbig")
        wq_trig = [nc.scalar, nc.gpsimd, nc.sync, nc.scalar]
        for c in range(4):
            wq_trig[c].dma_start(out=wTbig[:, 3072 * c:3072 * (c + 1)],
                                 in_=wTt[:, 3072 * c:3072 * (c + 1)])
        wT_sb = [wTbig[:, 768 * kk:768 * (kk + 1)] for kk in range(16)]
        gb_full = cpool.tile([128, 3 * CH], F32, tag="gb_full")
        nc.gpsimd.partition_broadcast(gb_full[:, 0:2 * CH], gb_row[:, 0:2 * CH])
        nc.gpsimd.partition_broadcast(gb_full[:, 2 * CH:3 * CH],
                                      gb_row[:, 2 * CH:3 * CH])
        bqkB = gb_full[:, 0:2 * CH]
        bvB = gb_full[:, 2 * CH:3 * CH]

        ident = cpool.tile([128, 128], BF, tag="ident")
        make_identity(nc, ident[:])
        ones_col = cpool.tile([128, 1], BF, tag="ones_col")
        nc.gpsimd.memset(ones_col[:], 1.0)
        ones_row = cpool.tile([1, 128], BF, tag="ones_row")
        nc.gpsimd.memset(ones_row[:], 1.0)

        # rope tables (gains folded in): frA, fiA, fiB, frB — resident.
        # First halves now; second halves + kwT after the ss DMA so the
        # tiny AllReduce input is not stuck behind bulk transfers.
        FH = 7 * 256
        ftab_sb = []
        for i in range(4):
            t = cpool.tile([128, NT * 256], BF, tag=f"ftab{i}", name=f"ftab{i}")
            nc.scalar.dma_start(out=t[:, 0:FH], in_=ftab[i][:, 0:FH])
            ftab_sb.append(t)

        # attention constants (cache halves of the window): tiles now,
        # DMAs deferred past the ss_in DMA
        kwT_sb = []
        cv_big = []
        cv_sb = [[], []]
        for hh in range(HPC):
            t = cpool.tile([128, TW], BF, tag=f"kwT{hh}", name=f"kwT{hh}")
            kwT_sb.append(t)
            big = cpool.tile([128, NT * D], BF, tag=f"cva{hh}", name=f"cva{hh}")
            cv_big.append(big)
            cv_sb[hh] = [big[:, j * D:(j + 1) * D] for j in range(NT)]

        qk_sb, v_sb, rq_store = [], [], {}
        for j in range(NT):
            qk_sb.append(ppool.tile([128, 2 * CH], F32, tag=f"qk{j}",
                                    name=f"qk{j}"))
            v_sb.append(cpool.tile([128, CH], BF, tag=f"v{j}", name=f"v{j}"))

        rqT_sb = [cpool.tile([128, S], BF, tag=f"rqT{hh}", name=f"rqT{hh}")
                  for hh in range(HPC)]
        oT_sb = [cpool.tile([128, S], BF, tag=f"oT{hh}", name=f"oT{hh}")
                 for hh in range(HPC)]

        ss = cpool.tile([128, 28], F32, tag="ss")
        nc.gpsimd.memset(ss[:], 0.0)
        eps_ap = cpool.tile([128, 1], F32, tag="eps_ap")
        nc.gpsimd.memset(eps_ap[:], EPS)

        rs_holder = {}

        def issue_ar():
            nc.scalar.dma_start(out=ss_in[:], in_=ss[:])
            nc.gpsimd.collective_compute(
                "AllReduce", Op.add, replica_groups=[core_ids],
                ins=[ss_in[:]], outs=[ss_out[:]])
            for i in range(4):
                nc.scalar.dma_start(out=ftab_sb[i][:, FH:],
                                    in_=ftab[i][:, FH:])
            for hh in range(HPC):
                nc.scalar.dma_start(out=kwT_sb[hh][:, 0:S], in_=ckT[hh])

        def finish_ar():
            ssg = cpool.tile([128, 28], F32, tag="ssg")
            nc.sync.dma_start(out=ssg[:], in_=ss_out[:])
            tmp = wpool.tile([128, 28], F32, tag="rstmp", name="rstmp")
            nc.scalar.activation(tmp[:], ssg[:], A.Ln, scale=1.0 / DIM,
                                 bias=eps_ap[:])
            rqk = cpool.tile([128, 28], F32, tag="rqk", name="rqk")
            nc.scalar.activation(rqk[:], tmp[:], A.Exp, scale=-0.5)
            rs_holder["rs_q"] = rqk[:, 0:14]
            rsk = cpool.tile([128, 14], F32, tag="rsk", name="rsk")
            nc.vector.tensor_scalar_mul(rsk[:], rqk[:, 14:28], SCALE)
            rs_holder["rs_k_s"] = rsk

        # rope on un-normalized q and k (rope commutes with the per-row rms
        # scale); per-channel gains are folded into the ftab tables
        def rope_gain(j, ktr_pool):
            off, sz = S_TILES[j]
            fA = ftab_sb[0][:sz, 256 * j:256 * (j + 1)].rearrange(
                "p (b c) -> p b c", b=4)
            fiA = ftab_sb[1][:sz, 256 * j:256 * (j + 1)].rearrange(
                "p (b c) -> p b c", b=4)
            fiB = ftab_sb[2][:sz, 256 * j:256 * (j + 1)].rearrange(
                "p (b c) -> p b c", b=4)
            frB = ftab_sb[3][:sz, 256 * j:256 * (j + 1)].rearrange(
                "p (b c) -> p b c", b=4)
            q3 = qk_sb[j][:sz, :].rearrange("p (b c) -> p b c", b=4)
            qe, qo = q3[:, :, 0:64], q3[:, :, 64:128]
            rq4 = rqpool.tile([128, 2 * CH], BF, tag=f"rq{j}", name=f"rq{j}")
            r3 = rq4[:sz, :].rearrange("p (b c) -> p b c", b=4)
            t1 = wpool.tile([128, 256], BF, tag="ropet1")
            t2 = wpool.tile([128, 256], BF, tag="ropet2")
            t13 = t1[:sz, :].rearrange("p (b c) -> p b c", b=4)
            t23 = t2[:sz, :].rearrange("p (b c) -> p b c", b=4)
            nc.vector.tensor_mul(t13, qe, fA)
            nc.vector.tensor_mul(t23, qo, fiA)
            nc.vector.tensor_sub(r3[:, :, 0:64], t13, t23)
            t3 = wpool.tile([128, 256], BF, tag="ropet1")
            t4 = wpool.tile([128, 256], BF, tag="ropet2")
            t33 = t3[:sz, :].rearrange("p (b c) -> p b c", b=4)
            t43 = t4[:sz, :].rearrange("p (b c) -> p b c", b=4)
            nc.vector.tensor_mul(t33, qe, fiB)
            nc.vector.tensor_mul(t43, qo, frB)
            nc.vector.tensor_add(r3[:, :, 64:128], t33, t43)
            rq_store[j] = rq4
            # k transposes now (pre-AllReduce): k-norm folds into exp scale
            for hh in range(HPC):
                tp = ktr_pool.tile([128, 128], BF, tag="ktr")
                nc.tensor.transpose(tp[:, :sz],
                                    rq4[:sz, CH + D * hh:CH + D * (hh + 1)],
                                    ident[:sz, :sz])
                nc.vector.tensor_copy(kwT_sb[hh][:, S + off:S + off + sz], tp[:, :sz])

        xt_trig = [nc.sync, nc.gpsimd]

        xt_ctr = [1]  # slot 0 is the prefetched xt0pre
        xt_keep = {}

        def load_xt(j, eng=None):
            n = xt_ctr[0]
            xt_ctr[0] += 1
            xt = wpool.tile([128, DIM], BF, tag=f"xt{n % 3}", bufs=1,
                            name=f"xt_{n}")
            (eng or xt_trig[j % 2]).dma_start(out=xt[:], in_=xT[j])
            return xt

        # ---- phase 1: q/k projection + sum-of-squares; then one AllReduce --
        with tc.tile_pool(name="pj", bufs=2, space="PSUM") as pj:
            for j, (off, sz) in enumerate(S_TILES):
                xt = xt0 if j == 0 else load_xt(j)
                if j >= 11:
                    xt_keep[j] = xt
                ps = pj.tile([128, 512], F32, tag="qk")
                for kk in range(16):
                    nc.tensor.matmul(ps[:sz, 0:512], xt[:, 128 * kk:128 * kk + sz],
                                     wT_sb[kk][:, 0:512],
                                     start=(kk == 0), stop=(kk == 15))
                nc.vector.tensor_add(qk_sb[j][:sz, :], ps[:sz, :], bqkB[:sz, :])
                sq = wpool.tile([128, CH], F32, tag="sqscratch", bufs=2)
                nc.scalar.activation(sq[:sz, :], qk_sb[j][:sz, 0:CH], A.Square,
                                     accum_out=ss[:sz, j:j + 1])
                sq2 = wpool.tile([128, CH], F32, tag="sqscratch", bufs=2)
                nc.scalar.activation(sq2[:sz, :], qk_sb[j][:sz, CH:2 * CH],
                                     A.Square, accum_out=ss[:sz, 14 + j:15 + j])
            issue_ar()
            # ---- phase 2: v projection + rope + k transposes (AR in flight);
            # runs j=13..0 so the last three phase-1 xt tiles are reused
            for j in list(range(NT - 1, -1, -1)):
                off, sz = S_TILES[j]
                xt = xt_keep[j] if j >= 11 else load_xt(j)
                ps = pj.tile([128, CH], F32, tag="v")
                for kk in range(16):
                    nc.tensor.matmul(ps[:sz, :], xt[:, 128 * kk:128 * kk + sz],
                                     wT_sb[kk][:, 512:768],
                                     start=(kk == 0), stop=(kk == 15))
                nc.vector.tensor_add(v_sb[j][:sz, :], ps[:sz, :], bvB[:sz, :])
                rope_gain(j, pj)

        wtpool.release()
        ppool.release()
        for hh in range(HPC):
            nc.sync.dma_start(out=cv_big[hh][:], in_=cv[hh])

        # ---- phase 3: q norm-scale + transposes, then attention ----
        with tc.tile_pool(name="pat", bufs=2, space="PSUM") as pat:
            finish_ar()
            rs_q = rs_holder["rs_q"]
            rs_k_s = rs_holder["rs_k_s"]
            for j, (off, sz) in enumerate(S_TILES):
                rqs = wpool.tile([128, CH], BF, tag="rqs", bufs=2,
                                 name=f"rqs{j}")
                nc.vector.tensor_scalar_mul(rqs[:sz, :],
                                            rq_store[j][:sz, 0:CH],
                                            rs_q[:sz, j:j + 1])
                for hh in range(HPC):
                    tp = pat.tile([128, 128], BF, tag="tr")
                    nc.tensor.transpose(tp[:, :sz],
                                        rqs[:sz, D * hh:D * (hh + 1)],
                                        ident[:sz, :sz])
                    nc.vector.tensor_copy(rqT_sb[hh][:, off:off + sz],
                                          tp[:, :sz])
            rqpool.release()
            # output-projection weights: big contiguous chunks on sync+gpsimd
            # (NOT the ACT queue — its triggers would stall attention exps)
            tpool = tc.alloc_tile_pool(name="tailp", bufs=1)
            woTbig = tpool.tile([128, 16 * 2048], BF, tag="woTbig",
                                name="woTbig")
            def load_woT(c):
                nc.gpsimd.dma_start(out=woTbig[:, 8192 * c:8192 * (c + 1)],
                                    in_=woTt[:, 8192 * c:8192 * (c + 1)])

            load_woT(0)
            load_woT(1)
            woT_sb = [woTbig[:, 2048 * kk:2048 * (kk + 1)] for kk in range(16)]

            att = {}

            def attn_state(hh, jc):
                st = att.get((hh, jc))
                if st is None:
                    o_ps = pat.tile([128, 880], F32, tag="o", bufs=1,
                                    name=f"o{hh}_{jc}")
                    den = wpool.tile([128, 880], BF, tag="den", bufs=2,
                                     name=f"den{hh}_{jc}")
                    pts = {}
                    st = att[(hh, jc)] = (o_ps, den, pts)
                return st

            def attn_sc(hh, jc, tlist):
                jof, jsz = SJ[jc]
                o_ps, den, pts = attn_state(hh, jc)
                for ti in tlist:
                    part, j2, toff, tsz = T_TILES[ti]
                    sc = pat.tile([128, 880], F32, tag="sc")
                    nc.tensor.matmul(
                        sc[:tsz, 0:512], kwT_sb[hh][:, toff:toff + tsz],
                        rqT_sb[hh][:, jof:jof + 512], start=True, stop=True)
                    nc.tensor.matmul(
                        sc[:tsz, 512:880], kwT_sb[hh][:, toff:toff + tsz],
                        rqT_sb[hh][:, jof + 512:jof + 880],
                        start=True, stop=True)
                    pT = wpool.tile([128, 880], BF, tag="pT", bufs=4)
                    esc = (SCALE if part == "c"
                           else rs_k_s[:tsz, j2:j2 + 1])
                    nc.scalar.activation(pT[:tsz, :], sc[:tsz, :], A.Exp,
                                         scale=esc)
                    if ti == 0:
                        nc.vector.tensor_copy(den[:, :], pT[:, :])
                    else:
                        nc.vector.tensor_add(den[:tsz, :], den[:tsz, :],
                                             pT[:tsz, :])
                    pts[ti] = pT

            def attn_pv(hh, jc, tlist):
                o_ps, den, pts = attn_state(hh, jc)
                for ti in tlist:
                    part, j2, toff, tsz = T_TILES[ti]
                    pT = pts.pop(ti)
                    vt = (cv_sb[hh][j2][:tsz, :] if part == "c"
                          else v_sb[j2][:tsz, D * hh:D * (hh + 1)])
                    last = ti == len(T_TILES) - 1
                    nc.tensor.matmul(o_ps[:, 0:512], vt, pT[:tsz, 0:512],
                                     start=(ti == 0), stop=last)
                    nc.tensor.matmul(o_ps[:, 512:880], vt, pT[:tsz, 512:880],
                                     start=(ti == 0), stop=last)

            def attn_finish(hh, jc, mid=None):
                jof, jsz = SJ[jc]
                o_ps, den, pts = att[(hh, jc)]
                # softmax denominator: PE partition-reduce, DVE approx
                # reciprocal, PE rank-1 broadcast, DVE normalize
                redps = pat.tile([128, 880], F32, tag="sc", name=f"red{hh}_{jc}")
                nc.tensor.matmul(redps[0:1, 0:512], ones_col[:, :],
                                 den[:, 0:512], start=True, stop=True)
                nc.tensor.matmul(redps[0:1, 512:880], ones_col[:, :],
                                 den[:, 512:880], start=True, stop=True)
                o_raw = wpool.tile([128, 880], BF, tag="oraw", bufs=2,
                                   name=f"oraw{hh}_{jc}")
                nc.vector.tensor_copy(o_raw[:, :jsz], o_ps[:, :jsz])
                drf = wpool.tile([1, 880], F32, tag="dln", bufs=1,
                                 name=f"dln{hh}_{jc}")
                nc.vector.reciprocal_approx_fast(drf[0:1, :], redps[0:1, :])
                denr = wpool.tile([1, 880], BF, tag="denr", bufs=2,
                                  name=f"denr{hh}_{jc}")
                nc.scalar.copy(denr[0:1, :], drf[0:1, :])
                if mid is not None:
                    mid()
                denb = pat.tile([128, 880], F32, tag="sc", name=f"denb{hh}_{jc}")
                nc.tensor.matmul(denb[:, 0:512], ones_row[:, :],
                                 denr[0:1, 0:512], start=True, stop=True)
                nc.tensor.matmul(denb[:, 512:880], ones_row[:, :],
                                 denr[0:1, 512:880], start=True, stop=True)
                nc.vector.tensor_mul(
                    oT_sb[hh][:, jof:jof + jsz], o_raw[:, :jsz], denb[:, :jsz])

            def emit_a2a(w, hh):
                nc.sync.dma_start(
                    out=a2a_in[w][hh][:].rearrange("d p s -> p d s"),
                    in_=oT_sb[hh][:, 880 * w:880 * (w + 1)]
                        .rearrange("p (d s) -> p d s", s=110))
                nc.gpsimd.collective_compute(
                    "AllToAll", mybir.AluOpType.bypass,
                    replica_groups=[core_ids],
                    ins=[a2a_in[w][hh][:]], outs=[a2a_out[w][hh][:]])

            otr_sb = {}

            def load_otr(w, hh):
                t = tpool.tile([128, 8 * 110], BF, tag=f"otr{w}_{hh}",
                               name=f"otr{w}_{hh}")
                nc.sync.dma_start(
                    out=t[:].rearrange("p (d s) -> p d s", s=110),
                    in_=a2a_out[w][hh][:].rearrange("d p s -> p d s"))
                otr_sb[(w, hh)] = t

            yp_store = {}

            def wave_y_mm(w, nlist, hhs, tags):
                for n in nlist:
                    yp = yp_store.get((w, n))
                    if yp is None:
                        yp = pat.tile([128, 512], F32, tag=tags[n],
                                      name=f"yp{w}_{n}")
                        yp_store[(w, n)] = yp
                    for kk in range(16):
                        src_c, hh = kk // 2, kk % 2
                        if hh not in hhs:
                            continue
                        nc.tensor.matmul(
                            yp[:110, :],
                            otr_sb[(w, hh)][:, 110 * src_c:110 * (src_c + 1)],
                            woT_sb[kk][:, 512 * n:512 * (n + 1)],
                            start=(kk == 0), stop=(kk == 15))

            def wave_y_chunk_out(w, n, yf):
                yp = yp_store[(w, n)]
                nc.scalar.copy(yf[:110, 512 * n:512 * (n + 1)], yp[:110, :])
                nc.sync.dma_start(
                    out=y_out[110 * w:110 * (w + 1), 512 * n:512 * (n + 1)],
                    in_=yf[:110, 512 * n:512 * (n + 1)])

            def wave_y_full(w):
                yf = wpool.tile([128, DIM], F32, tag="yf", bufs=1, name=f"yf{w}")
                for n in range(4):
                    wave_y_mm(w, [n], (0, 1), ["tr"] * 4)
                    wave_y_chunk_out(w, n, yf)

            def wave_y_out(w):
                yf = wpool.tile([128, DIM], F32, tag="yf", bufs=1, name=f"yf{w}")
                for n in range(4):
                    wave_y_chunk_out(w, n, yf)

            TRS = ["tr", "tr", "sc", "sc"]

            # chunk order: (0,0) (1,0) (0,1) (1,1); a2a emitted per
            # (wave, head) as soon as that head's wave chunk finishes
            attn_sc(0, 0, [0, 1])
            attn_pv(0, 0, [0, 1])
            for ti in range(2, 28):
                attn_sc(0, 0, [ti])
                attn_pv(0, 0, [ti])
            attn_sc(1, 0, [0, 1])
            attn_finish(0, 0)
            emit_a2a(0, 0)
            load_woT(2)
            attn_pv(1, 0, [0, 1])
            for ti in range(2, 28):
                attn_sc(1, 0, [ti])
                attn_pv(1, 0, [ti])
            attn_sc(0, 1, [0, 1])
            attn_finish(1, 0)
            emit_a2a(0, 1)
            load_woT(3)
            load_otr(0, 0)
            attn_pv(0, 1, [0, 1])
            for ti in range(2, 28):
                attn_sc(0, 1, [ti])
                attn_pv(0, 1, [ti])
            attn_sc(1, 1, [0, 1])
            attn_finish(0, 1)
            emit_a2a(1, 0)
            load_otr(0, 1)
            attn_pv(1, 1, [0, 1])
            for ti in range(2, 14):
                attn_sc(1, 1, [ti])
                attn_pv(1, 1, [ti])
            wave_y_full(0)
            load_otr(1, 0)
            for ti in range(14, 28):
                attn_sc(1, 1, [ti])
                attn_pv(1, 1, [ti])
            # last chunk: even-head y matmuls overlap the final AllToAll
            attn_finish(1, 1, mid=lambda: wave_y_mm(1, [0], (0,), TRS))
            emit_a2a(1, 1)
            wave_y_mm(1, [1, 2, 3], (0,), TRS)
            load_otr(1, 1)
            wave_y_mm(1, [0, 1, 2, 3], (1,), TRS)
            wave_y_out(1)
        tpool.release()


def _build():
    if "nc" not in _CACHE:
        nc = bacc.Bacc("TRN2", target_bir_lowering=False, debug=False,
                       num_devices=NCORES)
        _emit(nc)
        nc.compile()
        _CACHE["nc"] = nc
    return _CACHE["nc"]


def _make_fcomb(freqs):
    F, H, W = 2, 20, 44
    fr = np.asarray(freqs, np.float32)  # [1024, 64, 2]
    fpart = np.broadcast_to(fr[5:7, None, None, 0:22], (F, H, W, 22, 2))
    hpart = np.broadcast_to(fr[None, 0:H, None, 22:43], (F, H, W, 21, 2))
    wpart = np.broadcast_to(fr[None, None, 0:W, 43:64], (F, H, W, 21, 2))
    return np.concatenate([fpart, hpart, wpart], axis=3).reshape(S, 64, 2)


def _tile_major(a):
    """[S, C] -> [128, NT*C] tile-major (per-partition contiguous)."""
    C = a.shape[1]
    ap = np.zeros((NT * 128, C), np.float32)
    ap[:S] = a
    return np.ascontiguousarray(
        ap.reshape(NT, 128, C).transpose(1, 0, 2).reshape(128, NT * C))


def kernel(x, wq, bq, wk, bk, wv, bv, wo, bo, gq, gk, freqs, cache_k, cache_v):
    x = np.asarray(x, np.float32)
    wq, wk, wv, wo = (np.asarray(a, np.float32) for a in (wq, wk, wv, wo))
    bq, bk, bv, bo = (np.asarray(a, np.float32) for a in (bq, bk, bv, bo))
    gq, gk = np.asarray(gq, np.float32), np.asarray(gk, np.float32)
    cache_k = np.asarray(cache_k, np.float32)
    cache_v = np.asarray(cache_v, np.float32)

    fcomb = _make_fcomb(freqs)  # [S, 64, 2]
    fr_t, fi_t = fcomb[..., 0], fcomb[..., 1]  # [S, 64]
    # pre-tiled x^T: xT[j, p, kk*128+c] = x[128j+c, 128kk+p]
    xp = np.zeros((NT * 128, DIM), np.float32)
    xp[:S] = x[0]
    xT = np.ascontiguousarray(
        xp.reshape(NT, 128, 16, 128).transpose(0, 3, 2, 1).reshape(NT, 128, DIM)
    ).astype(BF16)

    # de-interleave rope channel pairs within each head: [2c] then [2c+1]
    # (applied consistently to wq/wk rows, their biases/gains, and the
    # transposed k-cache, so attention dot products are unchanged)
    perm = np.concatenate([np.arange(0, D, 2), np.arange(1, D, 2)])
    qk_perm = np.concatenate([h * D + perm for h in range(NH)])
    wqp, wkp = wq[qk_perm], wk[qk_perm]
    bqp, bkp = bq[qk_perm], bk[qk_perm]
    gqp, gkp = gq[qk_perm], gk[qk_perm]
    ck_perm = cache_k[0, WIN0:WIN0 + S][:, :, perm]  # [S, NH, D] channel-permuted

    woT_full = np.ascontiguousarray(wo.T).astype(np.float32)  # [DIM, DIM]
    in_maps = []
    for c in range(NCORES):
        hs = slice(CH * c, CH * (c + 1))
        h0 = HPC * c
        wTc = np.concatenate([wqp[hs].T, wkp[hs].T, wv[hs].T], axis=1)
        # tile-major: wTt[p, kk*768+cc] = wTc[128kk+p, cc]
        wTt = np.ascontiguousarray(
            wTc.reshape(16, 128, 768).transpose(1, 0, 2).reshape(128, 16 * 768)
        ).astype(BF16)
        woTt = np.ascontiguousarray(
            woT_full.reshape(16, 128, 2048).transpose(1, 0, 2)
            .reshape(128, 16 * 2048)).astype(BF16)
        ckTc = np.ascontiguousarray(
            ck_perm[:, h0:h0 + HPC, :].transpose(1, 2, 0)
        ).astype(BF16)  # [HPC, D, S]
        # pre-tiled cache-v: cvc[hh, p, j*128+d] = cv_window[128j+p, h, d]
        cw = np.zeros((NT * 128, HPC, D), np.float32)
        cw[:S] = cache_v[0, WIN0:WIN0 + S, h0:h0 + HPC, :]
        cvc = np.ascontiguousarray(
            cw.reshape(NT, 128, HPC, D).transpose(2, 1, 0, 3).reshape(HPC, 128, NT * D)
        ).astype(BF16)
        # rope tables with per-block gains folded in; blocks are
        # (q-h0, q-h1, k-h0, k-h1), each [64 even | 64 odd] channels
        ge = [gqp[hs][0:64], gqp[hs][128:192], gkp[hs][0:64], gkp[hs][128:192]]
        go = [gqp[hs][64:128], gqp[hs][192:256],
              gkp[hs][64:128], gkp[hs][192:256]]
        tabs = []
        for src, gl in ((fr_t, ge), (fi_t, go), (fi_t, ge), (fr_t, go)):
            tab = np.concatenate([src * gl[b][None, :] for b in range(4)],
                                 axis=1)  # [S, 256]
            tabs.append(_tile_major(tab).astype(BF16))
        gb = np.concatenate([bqp[hs], bkp[hs], bv[hs]])
        in_maps.append({
            "xT": xT, "wTt": wTt, "woTt": woTt,
            "ckT": ckTc, "cv": cvc,
            "ftab0": tabs[0], "ftab1": tabs[1], "ftab2": tabs[2],
            "ftab3": tabs[3],
            "gb": np.ascontiguousarray(gb)[None, :].astype(np.float32),
        })

    nc = _build()
    res = run_bass_kernel_spmd(nc, in_maps, list(range(NCORES)))
    _CACHE["last_result"] = res
    # all-to-all layout: core c returns rows [110c:110c+110] and
    # [880+110c:880+110c+110]
    y = np.empty((S, DIM), np.float32)
    for c in range(NCORES):
        yc = res.results[c]["y"]
        y[110 * c:110 * (c + 1)] = yc[:110]
        y[880 + 110 * c:880 + 110 * (c + 1)] = yc[110:]
    return (y + bo[None, :]).reshape(1, S, DIM).astype(np.float32)
